# revision 21
# baseline (speedup 1.0000x reference)
"""HGAT block on 8 Trainium2 NeuronCores (Bass/Tile, SPMD node-sharded).

Dense reformulation: the hypergraph incidence structure (he_nodes, he_edges)
is converted host-side into a dense per-core count matrix S[n_local, m]
(1024 x 1024), so every segment-sum becomes a dense matmul and the attention
softmax is computed on dense maps. Softmax denominators ride along as an
extra ones-column in the xt matmul. BatchNorm is shift-invariant per column,
so the conv biases bh1/bh2 drop out; BN affine corrections are applied
after the (pre-BN) he_attr matmul using he_attr(aff(h)) = aff(he_attr) with
the shift scaled by edge sizes Bm.

Sharding: nodes N=8192 split 1024/core. Per-edge partials (he_attr,
attention sums, eo aggregation) are all-reduced; BN stats ride as 2 extra
columns of the he_attr collectives for layers 1 and 2. x is loaded once in
bf16 (shared by the W1 matmul and the residual); output is stored bf16 and
upcast to f32 on host.

Master activations live in transposed layout hT[d=128 partitions, n=1024]
so BN affine/stats are per-partition ops; PE transposes flip orientation
where a matmul needs node-major operands.
"""

import sys
import types

import numpy as np

N, T, DM = 8192, 32, 128
M, NNZ = 1024, 131072
EPS = 1e-5
SLOPE = 0.2
NCORES = 8
NL = N // NCORES          # 1024 local nodes per core
NT = NL // 128            # 8 node tiles
MT = M // 128             # 8 edge tiles
KT = (T * DM) // 128      # 32 k-tiles for W1
D_IN = T * DM             # 4096

_PROGRAM = None


def _ensure_ntff_hook():
    try:
        import antenv.axon_hooks  # noqa: F401
        return
    except ImportError:
        pass
    try:
        import antenv
        from trn_agent_boot.trn_boot import _ntff_profile_via_ctypes
    except ImportError:
        return
    mod = types.ModuleType("antenv.axon_hooks")
    hook = _ntff_profile_via_ctypes("/opt/axon/libaxon_pjrt.so")
    mod.get_axon_ntff_profile_hook = lambda: hook
    mod.set_axon_ntff_profile_hook = lambda h: None
    sys.modules["antenv.axon_hooks"] = mod
    antenv.axon_hooks = mod


def build_program():
    from concourse import bacc, mybir, tile, masks

    f32 = mybir.dt.float32
    bf16 = mybir.dt.bfloat16
    AF = mybir.ActivationFunctionType
    ALU = mybir.AluOpType
    AX = mybir.AxisListType
    RG = [list(range(NCORES))]

    nc = bacc.Bacc("TRN2", target_bir_lowering=False, debug=False,
                   num_devices=NCORES)

    def din(name, shape, dt=f32):
        return nc.dram_tensor(name, list(shape), dt, kind="ExternalInput")

    xTb = din("xTb", [KT, 128, NL], bf16)
    S_nm = din("S_nm", [128, NT, M], bf16)
    W1b = din("W1b", [128, KT, DM], bf16)
    W3b = din("W3b", [DM, D_IN], bf16)
    Wh1b = din("Wh1b", [DM, 4 * DM], bf16)
    Wh2b = din("Wh2b", [DM, DM], bf16)
    wsx1 = din("wsx1", [DM, 4], bf16)
    wse1 = din("wse1", [DM, 4], bf16)
    wsx2 = din("wsx2", [DM, 1], bf16)
    wse2 = din("wse2", [DM, 1], bf16)
    b1T = din("b1T", [DM, 1])
    b3T = din("b3T", [DM, T])
    gbn = {k: din(k, [DM, 1]) for k in
           ("g1", "be1", "g2", "be2", "g3", "be3", "g4", "be4")}
    dinv1r = din("dinv1r", [1, NL], bf16)     # Dinv/heads as a row
    dinv2r = din("dinv2r", [1, NL], bf16)
    binvT = din("binvT", [128, MT])
    bmrow = din("bmrow", [1, M], bf16)
    outT = nc.dram_tensor("outT", [D_IN, NL], bf16, kind="ExternalOutput")

    GH = 2  # heads per attention group (bounds a0_em SBUF)

    he1_in = nc.dram_tensor("he1_in", [128, M + 2], f32)
    he1_out = nc.dram_tensor("he1_out", [128, M + 2], f32,
                             addr_space="Shared")
    he2_in = nc.dram_tensor("he2_in", [128, M + 2], f32)
    he2_out = nc.dram_tensor("he2_out", [128, M + 2], f32,
                             addr_space="Shared")
    eo1_in = nc.dram_tensor("eo1_in", [M, GH * 129], f32)
    eo1_out = nc.dram_tensor("eo1_out", [M, GH * 129], f32,
                             addr_space="Shared")
    eo2_in = nc.dram_tensor("eo2_in", [M, 129], f32)
    eo2_out = nc.dram_tensor("eo2_out", [M, 129], f32, addr_space="Shared")
    bn3_in = nc.dram_tensor("bn3_in", [128, 2], f32)
    bn3_out = nc.dram_tensor("bn3_out", [128, 2], f32)
    bn4_in = nc.dram_tensor("bn4_in", [128, 2], f32)
    bn4_out = nc.dram_tensor("bn4_out", [128, 2], f32)

    def allreduce(dst, src):
        nc.gpsimd.collective_compute(
            "AllReduce", ALU.add, replica_groups=RG,
            ins=[src[:].opt()], outs=[dst[:].opt()])

    with tile.TileContext(nc) as tc:
        with (
            tc.tile_pool(name="const", bufs=1) as constp,
            tc.tile_pool(name="state", bufs=1) as statep,
            tc.tile_pool(name="attn", bufs=1) as attnp,
            tc.tile_pool(name="big1", bufs=1) as big1p,
            tc.tile_pool(name="work", bufs=2) as workp,
            tc.tile_pool(name="xk", bufs=3) as xkp,
            tc.tile_pool(name="ps", bufs=4, space="PSUM") as psp,
            tc.tile_pool(name="ps_sm", bufs=4, space="PSUM") as pssm,
        ):
            ident = constp.tile([128, 128], bf16)
            masks.make_identity(nc, ident[:])

            def load_const(tag, shape, dt, src_ap, name=None):
                t = constp.tile(shape, dt, tag=tag, name=name or tag)
                nc.sync.dma_start(t[:], src_ap)
                return t

            # W1 and W3 share one slot (W3 loads after the W1 matmul);
            # the final-phase vB shares the S slot.
            w1_sb = load_const("wslot", [128, KT * DM], bf16,
                               W1b[:].rearrange("p k d -> p (k d)"),
                               name="w1sb")
            wh1_sb = load_const("wh1", [DM, 4 * DM], bf16, Wh1b[:])
            wh2_sb = load_const("wh2", [DM, DM], bf16, Wh2b[:])
            wsx1_sb = load_const("wsx1", [DM, 4], bf16, wsx1[:])
            wse1_sb = load_const("wse1", [DM, 4], bf16, wse1[:])
            wsx2_sb = load_const("wsx2", [DM, 1], bf16, wsx2[:])
            wse2_sb = load_const("wse2", [DM, 1], bf16, wse2[:])
            b1_sb = load_const("b1", [DM, 1], f32, b1T[:])
            b3_sb = load_const("b3", [DM, T], f32, b3T[:])
            gb_sb = {k: load_const(k, [DM, 1], f32, gbn[k][:]) for k in gbn}
            binv_sb = load_const("binv", [128, MT], f32, binvT[:])
            bm_row = load_const("bmr", [1, M], bf16, bmrow[:])
            d1_row = load_const("d1r", [1, NL], bf16, dinv1r[:])
            d2_row = load_const("d2r", [1, NL], bf16, dinv2r[:])
            eps_sb = constp.tile([128, 1], f32, tag="epsc")
            nc.gpsimd.memset(eps_sb[:], EPS)
            bm_bc = constp.tile([128, M], bf16, tag="bmbc")
            nc.gpsimd.partition_broadcast(bm_bc[:], bm_row[:1, :])
            dinv1_bc = constp.tile([128, NL], bf16, tag="d1bc")
            nc.gpsimd.partition_broadcast(dinv1_bc[:], d1_row[:1, :])
            dinv2_bc = constp.tile([128, NL], bf16, tag="d2bc")
            nc.gpsimd.partition_broadcast(dinv2_bc[:], d2_row[:1, :])
            s_sb = constp.tile([128, 2 * NT * M], bf16, tag="sslot",
                               name="ssb")
            nc.sync.dma_start(s_sb[:, 0:NT * M],
                              S_nm[:].rearrange("p n m -> p (n m)"))

            def s_tile(nt):
                return s_sb[:, nt * M:(nt + 1) * M]

            # transpose helper: quad-batched PE transposes, one DVE evac
            def transpose_cols(src_fn, dst, n128, dt=bf16):
                """dst[:, i*128:(i+1)*128] = src_fn(i).T for i in range(n128),
                batching 4 transposes per PSUM tile + single evac."""
                for q in range(0, n128, 4):
                    w = min(4, n128 - q)
                    trq = pssm.tile([128, 512], dt, tag="sm", name="trq")
                    for k in range(w):
                        nc.tensor.matmul(trq[:, k * 128:(k + 1) * 128],
                                         src_fn(q + k), ident[:],
                                         is_transpose=True)
                    nc.vector.tensor_copy(
                        dst[:, q * 128:(q + w) * 128], trq[:, 0:w * 128])

            # ======== h1 = lrelu(x @ W1 + b1), T-space ========
            hp = [psp.tile([128, 512], f32, tag="acc", name=f"w1p{i}")
                  for i in range(2)]
            for kt in range(KT):
                xk = xkp.tile([128, NL], bf16, tag="xk")
                nc.sync.dma_start(xk[:], xTb[kt, :, :])
                for i in range(2):
                    nc.tensor.matmul(
                        hp[i][:], w1_sb[:, kt * DM:(kt + 1) * DM],
                        xk[:, i * 512:(i + 1) * 512],
                        start=(kt == 0), stop=(kt == KT - 1))
            hT1 = statep.tile([128, NL], f32, tag="hT1")
            hT1_b = statep.tile([128, NL], bf16, tag="hT1b")
            for i in range(2):
                sl = slice(i * 512, (i + 1) * 512)
                nc.scalar.activation(hT1[:, sl], hp[i][:], AF.Prelu,
                                     bias=b1_sb[:, 0:1], alpha=SLOPE)
                nc.vector.tensor_copy(hT1_b[:, sl], hT1[:, sl])
            w3_sb = load_const("wslot", [DM, D_IN], bf16, W3b[:],
                               name="w3sb")

            ttr_dump = big1p.tile([128, NL], bf16, tag="ttrd")

            def stats_pair(st_ap, hT):
                """st cols 0/1 = sum(hT), sum(hT^2) along free axis."""
                nc.vector.reduce_sum(st_ap[:, 0:1], hT[:], axis=AX.X)
                nc.scalar.activation(ttr_dump[:], hT[:], AF.Square,
                                     accum_out=st_ap[:, 1:2])

            def bn_scales(sum_ap, sumsq_ap, g_sb, be_sb, count, tagp):
                sc = workp.tile([128, 1], f32, tag=f"sc{tagp}",
                                name=f"sc{tagp}")
                sh = workp.tile([128, 1], f32, tag=f"sh{tagp}",
                                name=f"sh{tagp}")
                tmp = workp.tile([128, 4], f32, tag="bnt", name=f"bnt{tagp}")
                mean, var, m2, rstd = (tmp[:, i:i + 1] for i in range(4))
                nc.scalar.mul(mean, sum_ap, 1.0 / count)
                nc.scalar.mul(var, sumsq_ap, 1.0 / count)
                nc.scalar.square(m2, mean)
                nc.vector.tensor_sub(var, var, m2)
                nc.scalar.activation(rstd, var, AF.Sqrt, bias=eps_sb[:, 0:1])
                nc.vector.reciprocal(rstd, rstd)
                nc.vector.tensor_mul(sc, g_sb[:], rstd)
                nc.vector.tensor_mul(sh, mean, sc)
                nc.vector.tensor_sub(sh, be_sb[:], sh)
                return sc, sh

            def bn_he_block(hT, hT_b, gk, bek, he_in, he_out, heT_b, tagp):
                """One allreduce carries the he_attr partial plus the BN
                stats (2 extra cols): a separate 1KB stats AR costs ~10-23us
                serialized ahead of this one on the CC stream."""
                # pre-BN node-major shadow for the he matmul
                hn_b = attnp.tile([128, NT * DM], bf16, tag="hnbn",
                                  name=f"hnbn{tagp}")
                transpose_cols(
                    lambda i: hT_b[:, i * 128:(i + 1) * 128], hn_b[:], NT)
                hep = [psp.tile([128, 512], f32, tag="acc", name=f"hep{i}")
                       for i in range(2)]
                for nt in range(NT):
                    for i in range(2):
                        nc.tensor.matmul(
                            hep[i][:], hn_b[:, nt * DM:(nt + 1) * DM],
                            s_tile(nt)[:, i * 512:(i + 1) * 512],
                            start=(nt == 0), stop=(nt == NT - 1))
                he_sb = big1p.tile([128, M + 2], f32, tag="ar",
                                   name=f"he{tagp}")
                for i in range(2):
                    nc.vector.tensor_copy(he_sb[:, i * 512:(i + 1) * 512],
                                          hep[i][:])
                stats_pair(he_sb[:, M:M + 2], hT)
                nc.sync.dma_start(he_in[:], he_sb[:])
                allreduce(he_out, he_in)
                str_ = workp.tile([128, 2], f32, tag="st", name=f"str{tagp}")
                nc.sync.dma_start(str_[:], he_out[:, M:M + 2])
                sc, sh = bn_scales(str_[:, 0:1], str_[:, 1:2],
                                   gb_sb[gk], gb_sb[bek], float(N), tagp)
                nc.vector.tensor_scalar(hT[:], hT[:], sc[:, 0:1], sh[:, 0:1],
                                        op0=ALU.mult, op1=ALU.add)
                nc.vector.tensor_copy(hT_b[:], hT[:])
                he_r = big1p.tile([128, M], f32, tag="ar", name=f"her{tagp}")
                nc.sync.dma_start(he_r[:], he_out[:, 0:M])
                heTf = big1p.tile([128, M], f32, tag="heTf",
                                  name=f"heTf{tagp}")
                nc.vector.tensor_scalar(heTf[:], he_r[:], sc[:, 0:1],
                                        None, op0=ALU.mult)
                nc.vector.scalar_tensor_tensor(
                    heTf[:], bm_bc[:], sh[:, 0:1], heTf[:],
                    op0=ALU.mult, op1=ALU.add)
                nc.vector.tensor_copy(heT_b[:], heTf[:])

            heT1_b = statep.tile([128, M], bf16, tag="heT1b")
            bn_he_block(hT1, hT1_b, "g1", "be1", he1_in, he1_out,
                        heT1_b, "1")

            def hconv(heads, hT, hT_b, heT_b, wh_sb, wsx_sb, wse_sb,
                      dinv_bc, eo_in, eo_out, hT_new, hT_new_b, lname):
                C = 129
                hn_b = attnp.tile([128, NT * DM], bf16, tag="hnbn",
                                  name=f"hnc{lname}")
                transpose_cols(
                    lambda i: hT_b[:, i * 128:(i + 1) * 128], hn_b[:], NT)
                ax = attnp.tile([128, NT * 4], f32, tag="ax",
                                name=f"ax{lname}")
                for nt in range(NT):
                    aps = pssm.tile([128, 4], f32, tag="sm", name="axp")
                    nc.tensor.matmul(aps[:, 0:heads],
                                     hn_b[:, nt * DM:(nt + 1) * DM],
                                     wsx_sb[:], start=True, stop=True)
                    nc.vector.tensor_copy(ax[:, nt * 4:nt * 4 + heads],
                                          aps[:, 0:heads])
                ae_rows = []
                for h in range(heads):
                    ae_row = attnp.tile([1, M], bf16, tag="aerow", bufs=4,
                                        name=f"aerow{lname}{h}")
                    for i in range(2):
                        aep = pssm.tile([1, 512], f32, tag="sm", name="aep")
                        nc.tensor.matmul(aep[:], wse_sb[:, h:h + 1],
                                         heT_b[:, i * 512:(i + 1) * 512],
                                         start=True, stop=True)
                        nc.vector.tensor_copy(
                            ae_row[0:1, i * 512:(i + 1) * 512], aep[:])
                    ae_rows.append(ae_row)
                noTacc = attnp.tile([128, NL], f32, tag="noacc",
                                    name=f"noacc{lname}")
                ngroups = (heads + GH - 1) // GH
                for g in range(ngroups):
                    ghs = list(range(g * GH, min(heads, (g + 1) * GH)))
                    gh = len(ghs)
                    xta = attnp.tile([128, NT * GH * C], bf16, tag="xta",
                                     name=f"xta{lname}{g}")
                    nc.gpsimd.memset(xta[:], 1.0)
                    for nt in range(NT):
                        xps = psp.tile([128, 512], f32, tag="acc",
                                       name="xtp")
                        nc.tensor.matmul(
                            xps[:, 0:gh * DM],
                            hn_b[:, nt * DM:(nt + 1) * DM],
                            wh_sb[:, ghs[0] * DM:(ghs[0] + gh) * DM],
                            start=True, stop=True)
                        base = nt * GH * C
                        for j in range(gh):
                            nc.vector.tensor_copy(
                                xta[:, base + j * C:base + j * C + DM],
                                xps[:, j * DM:(j + 1) * DM])
                    a0_em = attnp.tile([128, GH * MT * NL], bf16, tag="a0em",
                                       name=f"a0em{lname}{g}")
                    for j, h in enumerate(ghs):
                        ae_bc = workp.tile([128, M], bf16, tag="aebc",
                                           name="aebc")
                        nc.gpsimd.partition_broadcast(ae_bc[:],
                                                      ae_rows[h][0:1, :])
                        a0_nm = workp.tile([128, NT * M], bf16, tag="a0nm",
                                           name="a0nm")
                        for nt in range(NT):
                            nc.scalar.activation(
                                a0_nm[:, nt * M:(nt + 1) * M], ae_bc[:],
                                AF.Prelu,
                                bias=ax[:, nt * 4 + h:nt * 4 + h + 1],
                                alpha=SLOPE)
                        for nt in range(NT):
                            zt = a0_nm[:, nt * M:(nt + 1) * M]
                            nc.scalar.activation(zt, zt, AF.Exp)
                        for nt in range(NT):
                            zt = a0_nm[:, nt * M:(nt + 1) * M]
                            nc.vector.tensor_mul(zt, zt, s_tile(nt))
                        for mt in range(MT):
                            eps_ = psp.tile([128, C], f32, tag="acc",
                                            name="eop")
                            for nt in range(NT):
                                nc.tensor.matmul(
                                    eps_[:],
                                    a0_nm[:, nt * M + mt * 128:
                                          nt * M + (mt + 1) * 128],
                                    xta[:, (nt * GH + j) * C:
                                        (nt * GH + j + 1) * C],
                                    start=(nt == 0), stop=(nt == NT - 1))
                            eo_st = workp.tile([128, C], f32, tag="eost",
                                               name="eost")
                            nc.vector.tensor_copy(eo_st[:], eps_[:])
                            nc.sync.dma_start(
                                eo_in[mt * 128:(mt + 1) * 128,
                                      j * C:(j + 1) * C], eo_st[:])
                        for mt in range(MT):
                            st_ = (j * MT + mt) * NL
                            transpose_cols(
                                lambda i, _mt=mt: a0_nm[
                                    :, i * M + _mt * 128:
                                    i * M + (_mt + 1) * 128],
                                a0_em[:, st_:st_ + NL], NT)
                    allreduce(eo_out, eo_in)
                    eoH = attnp.tile([128, MT * GH * DM], bf16, tag="eoH",
                                     name=f"eoH{lname}{g}")
                    for mt in range(MT):
                        eor = workp.tile([128, gh * C], f32, tag="eor",
                                         name="eor", bufs=2)
                        nc.sync.dma_start(
                            eor[:], eo_out[mt * 128:(mt + 1) * 128,
                                           0:gh * C])
                        for j in range(gh):
                            sm = workp.tile([128, 2], f32, tag="smt",
                                            name="smt")
                            nc.vector.tensor_scalar(
                                sm[:, 0:1], eor[:, j * C + DM:j * C + DM + 1],
                                1e-30, None, op0=ALU.max)
                            nc.vector.reciprocal(sm[:, 0:1], sm[:, 0:1])
                            nc.vector.tensor_mul(sm[:, 1:2], sm[:, 0:1],
                                                 binv_sb[:, mt:mt + 1])
                            nc.vector.tensor_mul(sm[:, 1:2], sm[:, 1:2],
                                                 sm[:, 0:1])
                            nc.vector.tensor_scalar(
                                eoH[:, (mt * GH + j) * DM:
                                    (mt * GH + j + 1) * DM],
                                eor[:, j * C:j * C + DM], sm[:, 1:2], None,
                                op0=ALU.mult)
                    noT = [psp.tile([128, 512], f32, tag="acc",
                                    name=f"noT{lname}{g}{i}")
                           for i in range(2)]
                    for half in range(2):
                        k = 0
                        for j in range(gh):
                            for mt in range(MT):
                                st_ = (j * MT + mt) * NL
                                nc.tensor.matmul(
                                    noT[half][:],
                                    eoH[:, (mt * GH + j) * DM:
                                        (mt * GH + j + 1) * DM],
                                    a0_em[:, st_ + half * 512:
                                          st_ + (half + 1) * 512],
                                    start=(k == 0),
                                    stop=(k == gh * MT - 1))
                                k += 1
                        sl = slice(half * 512, (half + 1) * 512)
                        if g == 0:
                            nc.vector.tensor_copy(noTacc[:, sl],
                                                  noT[half][:])
                        else:
                            nc.vector.tensor_add(noTacc[:, sl],
                                                 noTacc[:, sl], noT[half][:])
                # residual epilogue in T-space: hT_new = hT + noTacc * dinv
                nsc = workp.tile([128, NL], f32, tag="nsc", name="nsc")
                nc.vector.tensor_mul(nsc[:], noTacc[:], dinv_bc[:])
                nc.vector.tensor_add(hT_new[:], hT[:], nsc[:])
                nc.vector.tensor_copy(hT_new_b[:], hT_new[:])

            hT2 = statep.tile([128, NL], f32, tag="hT2")
            hT2_b = statep.tile([128, NL], bf16, tag="hT2b")
            hconv(4, hT1, hT1_b, heT1_b, wh1_sb, wsx1_sb, wse1_sb,
                  dinv1_bc, eo1_in, eo1_out, hT2, hT2_b, "A")

            heT2_b = statep.tile([128, M], bf16, tag="heT2b")
            bn_he_block(hT2, hT2_b, "g2", "be2", he2_in, he2_out,
                        heT2_b, "2")

            hT3 = statep.tile([128, NL], f32, tag="hT3")
            hT3_b = statep.tile([128, NL], bf16, tag="hT3b")
            hconv(1, hT2, hT2_b, heT2_b, wh2_sb, wsx2_sb, wse2_sb,
                  dinv2_bc, eo2_in, eo2_out, hT3, hT3_b, "B")

            # ---- BN3 ----
            st3 = workp.tile([128, 2], f32, tag="st", name="st3")
            stats_pair(st3, hT3)
            nc.sync.dma_start(bn3_in[:], st3[:])
            allreduce(bn3_out, bn3_in)
            st3r = workp.tile([128, 2], f32, tag="st", name="st3r")
            nc.sync.dma_start(st3r[:], bn3_out[:])
            sc3, sh3 = bn_scales(st3r[:, 0:1], st3r[:, 1:2],
                                 gb_sb["g3"], gb_sb["be3"], float(N), "3")
            nc.vector.tensor_scalar(hT3[:], hT3[:], sc3[:, 0:1], sh3[:, 0:1],
                                    op0=ALU.mult, op1=ALU.add)
            nc.vector.tensor_copy(hT3_b[:], hT3[:])

            # ---- y = lrelu(h3 @ W3 + b3); v = x + y (bf16); BN4 ----
            vA = attnp.tile([128, (KT // 2) * NL], bf16, tag="a0em",
                            name="vA")
            vB = constp.tile([128, (KT // 2) * NL], bf16, tag="sslot",
                             name="vB")

            def v_slice(jc):
                t = vA if jc < KT // 2 else vB
                j = jc if jc < KT // 2 else jc - KT // 2
                return t[:, j * NL:(j + 1) * NL]

            bst = workp.tile([128, KT * 2 * 6], f32, tag="vsum",
                             name="bst")
            for jc in range(KT):
                xres = xkp.tile([128, NL], bf16, tag="xk", name="xres")
                nc.sync.dma_start(xres[:], xTb[jc, :, :])
                ytmp = big1p.tile([128, NL], bf16, tag="vtmp", name="ytmp",
                                  bufs=2)
                for i in range(2):
                    sl = slice(i * 512, (i + 1) * 512)
                    yps = psp.tile([128, 512], f32, tag="acc", name="yp")
                    nc.tensor.matmul(yps[:], w3_sb[:, jc * 128:(jc + 1) * 128],
                                     hT3_b[:, sl], start=True, stop=True)
                    nc.scalar.activation(ytmp[:, sl], yps[:], AF.Prelu,
                                         bias=b3_sb[:, jc:jc + 1], alpha=SLOPE)
                nc.gpsimd.tensor_add(v_slice(jc), ytmp[:], xres[:])
                for i in range(2):
                    nc.vector.bn_stats(
                        bst[:, (jc * 2 + i) * 6:(jc * 2 + i + 1) * 6],
                        v_slice(jc)[:, i * 512:(i + 1) * 512])
            # bn_aggr -> (mean, var); convert to (sum, sumsq) for the AR
            agg4 = workp.tile([128, 4], f32, tag="st", name="agg4")
            nc.vector.bn_aggr(agg4[:, 0:2], bst[:])
            st4s = workp.tile([128, 2], f32, tag="st", name="st4s")
            cnt = float(NL * KT)
            nc.scalar.mul(st4s[:, 0:1], agg4[:, 0:1], cnt)
            nc.scalar.square(agg4[:, 2:3], agg4[:, 0:1])
            nc.vector.tensor_add(agg4[:, 3:4], agg4[:, 1:2], agg4[:, 2:3])
            nc.scalar.mul(st4s[:, 1:2], agg4[:, 3:4], cnt)
            nc.sync.dma_start(bn4_in[:], st4s[:])
            allreduce(bn4_out, bn4_in)
            st4r = workp.tile([128, 2], f32, tag="st", name="st4r")
            nc.sync.dma_start(st4r[:], bn4_out[:])
            sc4, sh4 = bn_scales(st4r[:, 0:1], st4r[:, 1:2],
                                 gb_sb["g4"], gb_sb["be4"], float(N * T), "4")
            for jc in range(KT):
                ot = xkp.tile([128, NL], bf16, tag="xres", name="ot", bufs=3)
                nc.vector.tensor_scalar(ot[:], v_slice(jc),
                                        sc4[:, 0:1], sh4[:, 0:1],
                                        op0=ALU.mult, op1=ALU.add)
                nc.sync.dma_start(outT[jc * 128:(jc + 1) * 128, :], ot[:])

    nc.compile()
    return nc


def _prep_inputs(inputs):
    """Host-side preprocessing: shard, transpose, fold weights, build S."""
    x = np.ascontiguousarray(np.asarray(inputs["x"], np.float32))
    he_n = np.asarray(inputs["he_nodes"]).astype(np.int64)
    he_e = np.asarray(inputs["he_edges"]).astype(np.int64)
    W1 = np.asarray(inputs["W1"], np.float32)
    b1 = np.asarray(inputs["b1"], np.float32)
    Wh1 = np.asarray(inputs["Wh1"], np.float32)
    att1 = np.asarray(inputs["att1"], np.float32)
    Wh2 = np.asarray(inputs["Wh2"], np.float32)
    att2 = np.asarray(inputs["att2"], np.float32)
    W3 = np.asarray(inputs["W3"], np.float32)
    b3 = np.asarray(inputs["b3"], np.float32)

    try:
        import ml_dtypes
        bf = ml_dtypes.bfloat16
    except ImportError:  # pragma: no cover
        import jax.numpy as jnp
        bf = jnp.bfloat16

    S = np.zeros((M, N), np.float32)
    np.add.at(S, (he_e, he_n), 1.0)
    Dn = S.sum(axis=0)
    Bm = S.sum(axis=1)
    Dinv = np.where(Dn > 0, 1.0 / np.maximum(Dn, 1), 0.0).astype(np.float32)
    Binv = np.where(Bm > 0, 1.0 / np.maximum(Bm, 1), 0.0).astype(np.float32)

    def fold(Wh, att, heads):
        F = Wh.shape[1] // heads
        Whr = Wh.reshape(DM, heads, F)
        wx = np.einsum("dhf,hf->dh", Whr, att[0, :, :F]).astype(np.float32)
        we = np.einsum("dhf,hf->dh", Whr, att[0, :, F:]).astype(np.float32)
        return wx, we

    wx1, we1 = fold(Wh1, att1, 4)
    wx2, we2 = fold(Wh2, att2, 1)

    shared = {
        "W1b": np.ascontiguousarray(
            W1.reshape(KT, 128, DM).transpose(1, 0, 2)).astype(bf),
        "W3b": np.ascontiguousarray(W3).astype(bf),
        "Wh1b": np.ascontiguousarray(Wh1).astype(bf),
        "Wh2b": np.ascontiguousarray(Wh2).astype(bf),
        "wsx1": wx1.astype(bf), "wse1": we1.astype(bf),
        "wsx2": wx2.astype(bf), "wse2": we2.astype(bf),
        "b1T": b1.reshape(DM, 1),
        "b3T": np.ascontiguousarray(b3.reshape(T, DM).T),
        "binvT": np.ascontiguousarray(Binv.reshape(MT, 128).T),
        "bmrow": Bm.reshape(1, M).astype(bf),
    }
    for k in ("g1", "be1", "g2", "be2", "g3", "be3", "g4", "be4"):
        shared[k] = np.asarray(inputs[k], np.float32).reshape(DM, 1)

    in_maps = []
    for c in range(NCORES):
        rows = slice(c * NL, (c + 1) * NL)
        xT = np.ascontiguousarray(x[rows].reshape(NL, D_IN).T)  # [4096, NL]
        Sl = S[:, rows]                                          # [M, NL]
        S_nm = np.ascontiguousarray(
            Sl.T.reshape(NT, 128, M).transpose(1, 0, 2)).astype(bf)
        m = dict(shared)
        m["xTb"] = np.ascontiguousarray(
            xT.reshape(KT, 128, NL)).astype(bf)
        m["S_nm"] = S_nm
        m["dinv1r"] = (Dinv[rows] / 4.0).reshape(1, NL).astype(bf)
        m["dinv2r"] = Dinv[rows].reshape(1, NL).astype(bf)
        in_maps.append(m)
    return in_maps


def _run(inputs, trace=False, tmpdir=None):
    global _PROGRAM
    _ensure_ntff_hook()
    from concourse.bass_utils import run_bass_kernel_spmd

    if _PROGRAM is None:
        _PROGRAM = build_program()
    in_maps = _prep_inputs(inputs)
    res = run_bass_kernel_spmd(_PROGRAM, in_maps, list(range(NCORES)),
                               trace=trace, tmpdir=tmpdir)
    out = np.empty((N, T, DM), np.float32)
    for c in range(NCORES):
        oT = np.asarray(res.results[c]["outT"], np.float32)  # [4096, NL]
        out[c * NL:(c + 1) * NL] = oT.T.reshape(NL, T, DM)
    return out, res


def kernel(**inputs) -> np.ndarray:
    out, _ = _run(inputs)
    return out


if __name__ == "__main__":
    d = np.load("/root/problem/inputs.npz")
    inp = {k: d[k] for k in d.files}
    got = kernel(**inp)
    exp = np.load("/root/problem/expected.npy")
    denom = np.abs(exp).max()
    print("rel err:", np.abs(got - exp).max() / denom)



# revision 22
# speedup vs baseline: 1.0164x; 1.0164x over previous
"""HGAT block on 8 Trainium2 NeuronCores (Bass/Tile, SPMD node-sharded).

Dense reformulation: the hypergraph incidence structure (he_nodes, he_edges)
is converted host-side into a dense per-core count matrix S[n_local, m]
(1024 x 1024), so every segment-sum becomes a dense matmul and the attention
softmax is computed on dense maps. Softmax denominators ride along as an
extra ones-column in the xt matmul. BatchNorm is shift-invariant per column,
so the conv biases bh1/bh2 drop out; BN affine corrections are applied
after the (pre-BN) he_attr matmul using he_attr(aff(h)) = aff(he_attr) with
the shift scaled by edge sizes Bm.

Sharding: nodes N=8192 split 1024/core. Per-edge partials (he_attr,
attention sums, eo aggregation) are all-reduced; BN stats ride as 2 extra
columns of the he_attr collectives for layers 1 and 2. x is loaded once in
bf16 (shared by the W1 matmul and the residual); output is stored bf16 and
upcast to f32 on host.

Master activations live in transposed layout hT[d=128 partitions, n=1024]
so BN affine/stats are per-partition ops; PE transposes flip orientation
where a matmul needs node-major operands.
"""

import sys
import types

import numpy as np

N, T, DM = 8192, 32, 128
M, NNZ = 1024, 131072
EPS = 1e-5
SLOPE = 0.2
NCORES = 8
NL = N // NCORES          # 1024 local nodes per core
NT = NL // 128            # 8 node tiles
MT = M // 128             # 8 edge tiles
KT = (T * DM) // 128      # 32 k-tiles for W1
D_IN = T * DM             # 4096

_PROGRAM = None


def _ensure_ntff_hook():
    try:
        import antenv.axon_hooks  # noqa: F401
        return
    except ImportError:
        pass
    try:
        import antenv
        from trn_agent_boot.trn_boot import _ntff_profile_via_ctypes
    except ImportError:
        return
    mod = types.ModuleType("antenv.axon_hooks")
    hook = _ntff_profile_via_ctypes("/opt/axon/libaxon_pjrt.so")
    mod.get_axon_ntff_profile_hook = lambda: hook
    mod.set_axon_ntff_profile_hook = lambda h: None
    sys.modules["antenv.axon_hooks"] = mod
    antenv.axon_hooks = mod


def build_program():
    from concourse import bacc, mybir, tile, masks

    f32 = mybir.dt.float32
    bf16 = mybir.dt.bfloat16
    AF = mybir.ActivationFunctionType
    ALU = mybir.AluOpType
    AX = mybir.AxisListType
    RG = [list(range(NCORES))]

    nc = bacc.Bacc("TRN2", target_bir_lowering=False, debug=False,
                   num_devices=NCORES)

    def din(name, shape, dt=f32):
        return nc.dram_tensor(name, list(shape), dt, kind="ExternalInput")

    xTb = din("xTb", [KT, 128, NL], bf16)
    S_nm = din("S_nm", [128, NT, M], bf16)
    W1b = din("W1b", [128, KT, DM], bf16)
    W3b = din("W3b", [DM, D_IN], bf16)
    Wh1b = din("Wh1b", [DM, 4 * DM], bf16)
    Wh2b = din("Wh2b", [DM, DM], bf16)
    wsx1 = din("wsx1", [DM, 4], bf16)
    wse1 = din("wse1", [DM, 4], bf16)
    wsx2 = din("wsx2", [DM, 1], bf16)
    wse2 = din("wse2", [DM, 1], bf16)
    b1T = din("b1T", [DM, 1])
    b3T = din("b3T", [DM, T])
    gbn = {k: din(k, [DM, 1]) for k in
           ("g1", "be1", "g2", "be2", "g3", "be3", "g4", "be4")}
    dinv1r = din("dinv1r", [1, NL], bf16)     # Dinv/heads as a row
    dinv2r = din("dinv2r", [1, NL], bf16)
    binvT = din("binvT", [128, MT])
    bmrow = din("bmrow", [1, M], bf16)
    outT = nc.dram_tensor("outT", [D_IN, NL], bf16, kind="ExternalOutput")

    GH = 2  # heads per attention group (bounds a0_em SBUF)

    ars1_in = nc.dram_tensor("ars1_in", [128, 2], f32)
    ars1_out = nc.dram_tensor("ars1_out", [128, 2], f32)
    ars2_in = nc.dram_tensor("ars2_in", [128, 2], f32)
    ars2_out = nc.dram_tensor("ars2_out", [128, 2], f32)
    he1_in = nc.dram_tensor("he1_in", [128, M], f32)
    he1_out = nc.dram_tensor("he1_out", [128, M], f32, addr_space="Shared")
    he2_in = nc.dram_tensor("he2_in", [128, M], f32)
    he2_out = nc.dram_tensor("he2_out", [128, M], f32, addr_space="Shared")
    eo1_in = nc.dram_tensor("eo1_in", [M, GH * 129], f32)
    eo1_out = nc.dram_tensor("eo1_out", [M, GH * 129], f32,
                             addr_space="Shared")
    eo2_in = nc.dram_tensor("eo2_in", [M, 129], f32)
    eo2_out = nc.dram_tensor("eo2_out", [M, 129], f32, addr_space="Shared")
    bn3_in = nc.dram_tensor("bn3_in", [128, 2], f32)
    bn3_out = nc.dram_tensor("bn3_out", [128, 2], f32)
    bn4_in = nc.dram_tensor("bn4_in", [128, 2], f32)
    bn4_out = nc.dram_tensor("bn4_out", [128, 2], f32)

    def allreduce(dst, src):
        nc.gpsimd.collective_compute(
            "AllReduce", ALU.add, replica_groups=RG,
            ins=[src[:].opt()], outs=[dst[:].opt()])

    with tile.TileContext(nc) as tc:
        with (
            tc.tile_pool(name="const", bufs=1) as constp,
            tc.tile_pool(name="state", bufs=1) as statep,
            tc.tile_pool(name="attn", bufs=1) as attnp,
            tc.tile_pool(name="big1", bufs=1) as big1p,
            tc.tile_pool(name="work", bufs=2) as workp,
            tc.tile_pool(name="xk", bufs=3) as xkp,
            tc.tile_pool(name="ps", bufs=4, space="PSUM") as psp,
            tc.tile_pool(name="ps_sm", bufs=4, space="PSUM") as pssm,
        ):
            ident = constp.tile([128, 128], bf16)
            masks.make_identity(nc, ident[:])

            def load_const(tag, shape, dt, src_ap, name=None):
                t = constp.tile(shape, dt, tag=tag, name=name or tag)
                nc.sync.dma_start(t[:], src_ap)
                return t

            # W1 and W3 share one slot (W3 loads after the W1 matmul);
            # the final-phase vB shares the S slot.
            w1_sb = load_const("wslot", [128, KT * DM], bf16,
                               W1b[:].rearrange("p k d -> p (k d)"),
                               name="w1sb")
            wh1_sb = load_const("wh1", [DM, 4 * DM], bf16, Wh1b[:])
            wh2_sb = load_const("wh2", [DM, DM], bf16, Wh2b[:])
            wsx1_sb = load_const("wsx1", [DM, 4], bf16, wsx1[:])
            wse1_sb = load_const("wse1", [DM, 4], bf16, wse1[:])
            wsx2_sb = load_const("wsx2", [DM, 1], bf16, wsx2[:])
            wse2_sb = load_const("wse2", [DM, 1], bf16, wse2[:])
            b1_sb = load_const("b1", [DM, 1], f32, b1T[:])
            b3_sb = load_const("b3", [DM, T], f32, b3T[:])
            gb_sb = {k: load_const(k, [DM, 1], f32, gbn[k][:]) for k in gbn}
            binv_sb = load_const("binv", [128, MT], f32, binvT[:])
            bm_row = load_const("bmr", [1, M], bf16, bmrow[:])
            d1_row = load_const("d1r", [1, NL], bf16, dinv1r[:])
            d2_row = load_const("d2r", [1, NL], bf16, dinv2r[:])
            eps_sb = constp.tile([128, 1], f32, tag="epsc")
            nc.gpsimd.memset(eps_sb[:], EPS)
            bm_bc = constp.tile([128, M], bf16, tag="bmbc")
            nc.gpsimd.partition_broadcast(bm_bc[:], bm_row[:1, :])
            dinv1_bc = constp.tile([128, NL], bf16, tag="d1bc")
            nc.gpsimd.partition_broadcast(dinv1_bc[:], d1_row[:1, :])
            dinv2_bc = constp.tile([128, NL], bf16, tag="d2bc")
            nc.gpsimd.partition_broadcast(dinv2_bc[:], d2_row[:1, :])
            s_sb = constp.tile([128, 2 * NT * M], bf16, tag="sslot",
                               name="ssb")
            nc.sync.dma_start(s_sb[:, 0:NT * M],
                              S_nm[:].rearrange("p n m -> p (n m)"))

            def s_tile(nt):
                return s_sb[:, nt * M:(nt + 1) * M]

            # transpose helper: quad-batched PE transposes, one DVE evac
            def transpose_cols(src_fn, dst, n128, dt=bf16):
                """dst[:, i*128:(i+1)*128] = src_fn(i).T for i in range(n128),
                batching 4 transposes per PSUM tile + single evac."""
                for q in range(0, n128, 4):
                    w = min(4, n128 - q)
                    trq = pssm.tile([128, 512], dt, tag="sm", name="trq")
                    for k in range(w):
                        nc.tensor.matmul(trq[:, k * 128:(k + 1) * 128],
                                         src_fn(q + k), ident[:],
                                         is_transpose=True)
                    nc.vector.tensor_copy(
                        dst[:, q * 128:(q + w) * 128], trq[:, 0:w * 128])

            # ======== h1 = lrelu(x @ W1 + b1), T-space ========
            hp = [psp.tile([128, 512], f32, tag="acc", name=f"w1p{i}")
                  for i in range(2)]
            for kt in range(KT):
                xk = xkp.tile([128, NL], bf16, tag="xk")
                nc.sync.dma_start(xk[:], xTb[kt, :, :])
                for i in range(2):
                    nc.tensor.matmul(
                        hp[i][:], w1_sb[:, kt * DM:(kt + 1) * DM],
                        xk[:, i * 512:(i + 1) * 512],
                        start=(kt == 0), stop=(kt == KT - 1))
            hT1 = statep.tile([128, NL], f32, tag="hT1")
            hT1_b = statep.tile([128, NL], bf16, tag="hT1b")
            for i in range(2):
                sl = slice(i * 512, (i + 1) * 512)
                nc.scalar.activation(hT1[:, sl], hp[i][:], AF.Prelu,
                                     bias=b1_sb[:, 0:1], alpha=SLOPE)
                nc.vector.tensor_copy(hT1_b[:, sl], hT1[:, sl])
            w3_sb = load_const("wslot", [DM, D_IN], bf16, W3b[:],
                               name="w3sb")

            ttr_dump = big1p.tile([128, NL], bf16, tag="ttrd")

            def stats_pair(st_ap, hT):
                """st cols 0/1 = sum(hT), sum(hT^2) along free axis."""
                nc.vector.reduce_sum(st_ap[:, 0:1], hT[:], axis=AX.X)
                nc.scalar.activation(ttr_dump[:], hT[:], AF.Square,
                                     accum_out=st_ap[:, 1:2])

            def bn_scales(sum_ap, sumsq_ap, g_sb, be_sb, count, tagp):
                sc = workp.tile([128, 1], f32, tag=f"sc{tagp}",
                                name=f"sc{tagp}")
                sh = workp.tile([128, 1], f32, tag=f"sh{tagp}",
                                name=f"sh{tagp}")
                tmp = workp.tile([128, 4], f32, tag="bnt", name=f"bnt{tagp}")
                mean, var, m2, rstd = (tmp[:, i:i + 1] for i in range(4))
                nc.scalar.mul(mean, sum_ap, 1.0 / count)
                nc.scalar.mul(var, sumsq_ap, 1.0 / count)
                nc.scalar.square(m2, mean)
                nc.vector.tensor_sub(var, var, m2)
                nc.scalar.activation(rstd, var, AF.Sqrt, bias=eps_sb[:, 0:1])
                nc.vector.reciprocal(rstd, rstd)
                nc.vector.tensor_mul(sc, g_sb[:], rstd)
                nc.vector.tensor_mul(sh, mean, sc)
                nc.vector.tensor_sub(sh, be_sb[:], sh)
                return sc, sh

            def bn_he_block(hT, hT_b, gk, bek, ars_in, ars_out, he_in,
                            he_out, heT_b, tagp):
                """Split collectives: stats allreduce (small, gates affine)
                runs first; the he_attr partial allreduce overlaps with the
                affine + downstream transposes."""
                st = workp.tile([128, 2], f32, tag="st", name=f"st{tagp}")
                stats_pair(st, hT)
                nc.sync.dma_start(ars_in[:], st[:])
                allreduce(ars_out, ars_in)
                # pre-BN node-major shadow for the he matmul
                hn_b = attnp.tile([128, NT * DM], bf16, tag="hnbn",
                                  name=f"hnbn{tagp}")
                transpose_cols(
                    lambda i: hT_b[:, i * 128:(i + 1) * 128], hn_b[:], NT)
                hep = [psp.tile([128, 512], f32, tag="acc", name=f"hep{i}")
                       for i in range(2)]
                for nt in range(NT):
                    for i in range(2):
                        nc.tensor.matmul(
                            hep[i][:], hn_b[:, nt * DM:(nt + 1) * DM],
                            s_tile(nt)[:, i * 512:(i + 1) * 512],
                            start=(nt == 0), stop=(nt == NT - 1))
                he_sb = big1p.tile([128, M], f32, tag="ar", name=f"he{tagp}")
                for i in range(2):
                    nc.vector.tensor_copy(he_sb[:, i * 512:(i + 1) * 512],
                                          hep[i][:])
                nc.sync.dma_start(he_in[:], he_sb[:])
                allreduce(he_out, he_in)
                str_ = workp.tile([128, 2], f32, tag="st", name=f"str{tagp}")
                nc.sync.dma_start(str_[:], ars_out[:])
                sc, sh = bn_scales(str_[:, 0:1], str_[:, 1:2],
                                   gb_sb[gk], gb_sb[bek], float(N), tagp)
                nc.vector.tensor_scalar(hT[:], hT[:], sc[:, 0:1], sh[:, 0:1],
                                        op0=ALU.mult, op1=ALU.add)
                nc.vector.tensor_copy(hT_b[:], hT[:])
                he_r = big1p.tile([128, M], f32, tag="ar", name=f"her{tagp}")
                nc.sync.dma_start(he_r[:], he_out[:])
                heTf = big1p.tile([128, M], f32, tag="heTf",
                                  name=f"heTf{tagp}")
                nc.vector.tensor_scalar(heTf[:], he_r[:], sc[:, 0:1],
                                        None, op0=ALU.mult)
                nc.vector.scalar_tensor_tensor(
                    heTf[:], bm_bc[:], sh[:, 0:1], heTf[:],
                    op0=ALU.mult, op1=ALU.add)
                nc.vector.tensor_copy(heT_b[:], heTf[:])

            heT1_b = statep.tile([128, M], bf16, tag="heT1b")
            bn_he_block(hT1, hT1_b, "g1", "be1", ars1_in, ars1_out,
                        he1_in, he1_out, heT1_b, "1")

            def hconv(heads, hT, hT_b, heT_b, wh_sb, wsx_sb, wse_sb,
                      dinv_bc, eo_in, eo_out, hT_new, hT_new_b, lname):
                C = 129
                hn_b = attnp.tile([128, NT * DM], bf16, tag="hnbn",
                                  name=f"hnc{lname}")
                transpose_cols(
                    lambda i: hT_b[:, i * 128:(i + 1) * 128], hn_b[:], NT)
                ax = attnp.tile([128, NT * 4], f32, tag="ax",
                                name=f"ax{lname}")
                for nt in range(NT):
                    aps = pssm.tile([128, 4], f32, tag="sm", name="axp")
                    nc.tensor.matmul(aps[:, 0:heads],
                                     hn_b[:, nt * DM:(nt + 1) * DM],
                                     wsx_sb[:], start=True, stop=True)
                    nc.vector.tensor_copy(ax[:, nt * 4:nt * 4 + heads],
                                          aps[:, 0:heads])
                ae_rows = []
                for h in range(heads):
                    ae_row = attnp.tile([1, M], bf16, tag="aerow", bufs=4,
                                        name=f"aerow{lname}{h}")
                    for i in range(2):
                        aep = pssm.tile([1, 512], f32, tag="sm", name="aep")
                        nc.tensor.matmul(aep[:], wse_sb[:, h:h + 1],
                                         heT_b[:, i * 512:(i + 1) * 512],
                                         start=True, stop=True)
                        nc.vector.tensor_copy(
                            ae_row[0:1, i * 512:(i + 1) * 512], aep[:])
                    ae_rows.append(ae_row)
                noTacc = attnp.tile([128, NL], f32, tag="noacc",
                                    name=f"noacc{lname}")
                ngroups = (heads + GH - 1) // GH
                for g in range(ngroups):
                    ghs = list(range(g * GH, min(heads, (g + 1) * GH)))
                    gh = len(ghs)
                    xta = attnp.tile([128, NT * GH * C], bf16, tag="xta",
                                     name=f"xta{lname}{g}")
                    nc.gpsimd.memset(xta[:], 1.0)
                    for nt in range(NT):
                        xps = psp.tile([128, 512], f32, tag="acc",
                                       name="xtp")
                        nc.tensor.matmul(
                            xps[:, 0:gh * DM],
                            hn_b[:, nt * DM:(nt + 1) * DM],
                            wh_sb[:, ghs[0] * DM:(ghs[0] + gh) * DM],
                            start=True, stop=True)
                        base = nt * GH * C
                        for j in range(gh):
                            nc.vector.tensor_copy(
                                xta[:, base + j * C:base + j * C + DM],
                                xps[:, j * DM:(j + 1) * DM])
                    a0_em = attnp.tile([128, GH * MT * NL], bf16, tag="a0em",
                                       name=f"a0em{lname}{g}")
                    for j, h in enumerate(ghs):
                        ae_bc = workp.tile([128, M], bf16, tag="aebc",
                                           name="aebc")
                        nc.gpsimd.partition_broadcast(ae_bc[:],
                                                      ae_rows[h][0:1, :])
                        a0_nm = workp.tile([128, NT * M], bf16, tag="a0nm",
                                           name="a0nm")
                        for nt in range(NT):
                            nc.scalar.activation(
                                a0_nm[:, nt * M:(nt + 1) * M], ae_bc[:],
                                AF.Prelu,
                                bias=ax[:, nt * 4 + h:nt * 4 + h + 1],
                                alpha=SLOPE)
                        for nt in range(NT):
                            zt = a0_nm[:, nt * M:(nt + 1) * M]
                            nc.scalar.activation(zt, zt, AF.Exp)
                        for nt in range(NT):
                            zt = a0_nm[:, nt * M:(nt + 1) * M]
                            nc.vector.tensor_mul(zt, zt, s_tile(nt))
                        for mt in range(MT):
                            eps_ = psp.tile([128, C], f32, tag="acc",
                                            name="eop")
                            for nt in range(NT):
                                nc.tensor.matmul(
                                    eps_[:],
                                    a0_nm[:, nt * M + mt * 128:
                                          nt * M + (mt + 1) * 128],
                                    xta[:, (nt * GH + j) * C:
                                        (nt * GH + j + 1) * C],
                                    start=(nt == 0), stop=(nt == NT - 1))
                            eo_st = workp.tile([128, C], f32, tag="eost",
                                               name="eost")
                            nc.vector.tensor_copy(eo_st[:], eps_[:])
                            nc.sync.dma_start(
                                eo_in[mt * 128:(mt + 1) * 128,
                                      j * C:(j + 1) * C], eo_st[:])
                        for mt in range(MT):
                            st_ = (j * MT + mt) * NL
                            transpose_cols(
                                lambda i, _mt=mt: a0_nm[
                                    :, i * M + _mt * 128:
                                    i * M + (_mt + 1) * 128],
                                a0_em[:, st_:st_ + NL], NT)
                    allreduce(eo_out, eo_in)
                    eoH = attnp.tile([128, MT * GH * DM], bf16, tag="eoH",
                                     name=f"eoH{lname}{g}")
                    for mt in range(MT):
                        eor = workp.tile([128, gh * C], f32, tag="eor",
                                         name="eor", bufs=2)
                        nc.sync.dma_start(
                            eor[:], eo_out[mt * 128:(mt + 1) * 128,
                                           0:gh * C])
                        for j in range(gh):
                            sm = workp.tile([128, 2], f32, tag="smt",
                                            name="smt")
                            nc.vector.tensor_scalar(
                                sm[:, 0:1], eor[:, j * C + DM:j * C + DM + 1],
                                1e-30, None, op0=ALU.max)
                            nc.vector.reciprocal(sm[:, 0:1], sm[:, 0:1])
                            nc.vector.tensor_mul(sm[:, 1:2], sm[:, 0:1],
                                                 binv_sb[:, mt:mt + 1])
                            nc.vector.tensor_mul(sm[:, 1:2], sm[:, 1:2],
                                                 sm[:, 0:1])
                            nc.vector.tensor_scalar(
                                eoH[:, (mt * GH + j) * DM:
                                    (mt * GH + j + 1) * DM],
                                eor[:, j * C:j * C + DM], sm[:, 1:2], None,
                                op0=ALU.mult)
                    noT = [psp.tile([128, 512], f32, tag="acc",
                                    name=f"noT{lname}{g}{i}")
                           for i in range(2)]
                    for half in range(2):
                        k = 0
                        for j in range(gh):
                            for mt in range(MT):
                                st_ = (j * MT + mt) * NL
                                nc.tensor.matmul(
                                    noT[half][:],
                                    eoH[:, (mt * GH + j) * DM:
                                        (mt * GH + j + 1) * DM],
                                    a0_em[:, st_ + half * 512:
                                          st_ + (half + 1) * 512],
                                    start=(k == 0),
                                    stop=(k == gh * MT - 1))
                                k += 1
                        sl = slice(half * 512, (half + 1) * 512)
                        if g == 0:
                            nc.vector.tensor_copy(noTacc[:, sl],
                                                  noT[half][:])
                        else:
                            nc.vector.tensor_add(noTacc[:, sl],
                                                 noTacc[:, sl], noT[half][:])
                # residual epilogue in T-space: hT_new = hT + noTacc * dinv
                nsc = workp.tile([128, NL], f32, tag="nsc", name="nsc")
                nc.vector.tensor_mul(nsc[:], noTacc[:], dinv_bc[:])
                nc.vector.tensor_add(hT_new[:], hT[:], nsc[:])
                nc.vector.tensor_copy(hT_new_b[:], hT_new[:])

            hT2 = statep.tile([128, NL], f32, tag="hT2")
            hT2_b = statep.tile([128, NL], bf16, tag="hT2b")
            hconv(4, hT1, hT1_b, heT1_b, wh1_sb, wsx1_sb, wse1_sb,
                  dinv1_bc, eo1_in, eo1_out, hT2, hT2_b, "A")

            heT2_b = statep.tile([128, M], bf16, tag="heT2b")
            bn_he_block(hT2, hT2_b, "g2", "be2", ars2_in, ars2_out,
                        he2_in, he2_out, heT2_b, "2")

            hT3 = statep.tile([128, NL], f32, tag="hT3")
            hT3_b = statep.tile([128, NL], bf16, tag="hT3b")
            hconv(1, hT2, hT2_b, heT2_b, wh2_sb, wsx2_sb, wse2_sb,
                  dinv2_bc, eo2_in, eo2_out, hT3, hT3_b, "B")

            # ---- BN3 ----
            st3 = workp.tile([128, 2], f32, tag="st", name="st3")
            stats_pair(st3, hT3)
            nc.sync.dma_start(bn3_in[:], st3[:])
            allreduce(bn3_out, bn3_in)
            st3r = workp.tile([128, 2], f32, tag="st", name="st3r")
            nc.sync.dma_start(st3r[:], bn3_out[:])
            sc3, sh3 = bn_scales(st3r[:, 0:1], st3r[:, 1:2],
                                 gb_sb["g3"], gb_sb["be3"], float(N), "3")
            nc.vector.tensor_scalar(hT3[:], hT3[:], sc3[:, 0:1], sh3[:, 0:1],
                                    op0=ALU.mult, op1=ALU.add)
            nc.vector.tensor_copy(hT3_b[:], hT3[:])

            # ---- y = lrelu(h3 @ W3 + b3); v = x + y (bf16); BN4 ----
            vA = attnp.tile([128, (KT // 2) * NL], bf16, tag="a0em",
                            name="vA")
            vB = constp.tile([128, (KT // 2) * NL], bf16, tag="sslot",
                             name="vB")

            def v_slice(jc):
                t = vA if jc < KT // 2 else vB
                j = jc if jc < KT // 2 else jc - KT // 2
                return t[:, j * NL:(j + 1) * NL]

            vsum = workp.tile([128, KT], f32, tag="vsum", name="vsum")
            vsq = workp.tile([128, KT], f32, tag="vsq", name="vsq")
            for jc in range(KT):
                xres = xkp.tile([128, NL], bf16, tag="xk", name="xres")
                nc.sync.dma_start(xres[:], xTb[jc, :, :])
                ytmp = big1p.tile([128, NL], bf16, tag="vtmp", name="ytmp",
                                  bufs=2)
                for i in range(2):
                    sl = slice(i * 512, (i + 1) * 512)
                    yps = psp.tile([128, 512], f32, tag="acc", name="yp")
                    nc.tensor.matmul(yps[:], w3_sb[:, jc * 128:(jc + 1) * 128],
                                     hT3_b[:, sl], start=True, stop=True)
                    nc.scalar.activation(ytmp[:, sl], yps[:], AF.Prelu,
                                         bias=b3_sb[:, jc:jc + 1], alpha=SLOPE)
                nc.vector.tensor_add(v_slice(jc), ytmp[:], xres[:])
                nc.vector.reduce_sum(vsum[:, jc:jc + 1], v_slice(jc),
                                     axis=AX.X)
                nc.scalar.activation(ttr_dump[:], v_slice(jc), AF.Square,
                                     accum_out=vsq[:, jc:jc + 1])
            st4s = workp.tile([128, 2], f32, tag="st", name="st4s")
            nc.vector.reduce_sum(st4s[:, 0:1], vsum[:], axis=AX.X)
            nc.vector.reduce_sum(st4s[:, 1:2], vsq[:], axis=AX.X)
            nc.sync.dma_start(bn4_in[:], st4s[:])
            allreduce(bn4_out, bn4_in)
            st4r = workp.tile([128, 2], f32, tag="st", name="st4r")
            nc.sync.dma_start(st4r[:], bn4_out[:])
            sc4, sh4 = bn_scales(st4r[:, 0:1], st4r[:, 1:2],
                                 gb_sb["g4"], gb_sb["be4"], float(N * T), "4")
            for jc in range(KT):
                ot = xkp.tile([128, NL], bf16, tag="xres", name="ot", bufs=3)
                nc.vector.tensor_scalar(ot[:], v_slice(jc),
                                        sc4[:, 0:1], sh4[:, 0:1],
                                        op0=ALU.mult, op1=ALU.add)
                nc.sync.dma_start(outT[jc * 128:(jc + 1) * 128, :], ot[:])

    nc.compile()
    return nc


def _prep_inputs(inputs):
    """Host-side preprocessing: shard, transpose, fold weights, build S."""
    x = np.ascontiguousarray(np.asarray(inputs["x"], np.float32))
    he_n = np.asarray(inputs["he_nodes"]).astype(np.int64)
    he_e = np.asarray(inputs["he_edges"]).astype(np.int64)
    W1 = np.asarray(inputs["W1"], np.float32)
    b1 = np.asarray(inputs["b1"], np.float32)
    Wh1 = np.asarray(inputs["Wh1"], np.float32)
    att1 = np.asarray(inputs["att1"], np.float32)
    Wh2 = np.asarray(inputs["Wh2"], np.float32)
    att2 = np.asarray(inputs["att2"], np.float32)
    W3 = np.asarray(inputs["W3"], np.float32)
    b3 = np.asarray(inputs["b3"], np.float32)

    try:
        import ml_dtypes
        bf = ml_dtypes.bfloat16
    except ImportError:  # pragma: no cover
        import jax.numpy as jnp
        bf = jnp.bfloat16

    S = np.zeros((M, N), np.float32)
    np.add.at(S, (he_e, he_n), 1.0)
    Dn = S.sum(axis=0)
    Bm = S.sum(axis=1)
    Dinv = np.where(Dn > 0, 1.0 / np.maximum(Dn, 1), 0.0).astype(np.float32)
    Binv = np.where(Bm > 0, 1.0 / np.maximum(Bm, 1), 0.0).astype(np.float32)

    def fold(Wh, att, heads):
        F = Wh.shape[1] // heads
        Whr = Wh.reshape(DM, heads, F)
        wx = np.einsum("dhf,hf->dh", Whr, att[0, :, :F]).astype(np.float32)
        we = np.einsum("dhf,hf->dh", Whr, att[0, :, F:]).astype(np.float32)
        return wx, we

    wx1, we1 = fold(Wh1, att1, 4)
    wx2, we2 = fold(Wh2, att2, 1)

    shared = {
        "W1b": np.ascontiguousarray(
            W1.reshape(KT, 128, DM).transpose(1, 0, 2)).astype(bf),
        "W3b": np.ascontiguousarray(W3).astype(bf),
        "Wh1b": np.ascontiguousarray(Wh1).astype(bf),
        "Wh2b": np.ascontiguousarray(Wh2).astype(bf),
        "wsx1": wx1.astype(bf), "wse1": we1.astype(bf),
        "wsx2": wx2.astype(bf), "wse2": we2.astype(bf),
        "b1T": b1.reshape(DM, 1),
        "b3T": np.ascontiguousarray(b3.reshape(T, DM).T),
        "binvT": np.ascontiguousarray(Binv.reshape(MT, 128).T),
        "bmrow": Bm.reshape(1, M).astype(bf),
    }
    for k in ("g1", "be1", "g2", "be2", "g3", "be3", "g4", "be4"):
        shared[k] = np.asarray(inputs[k], np.float32).reshape(DM, 1)

    in_maps = []
    for c in range(NCORES):
        rows = slice(c * NL, (c + 1) * NL)
        xT = np.ascontiguousarray(x[rows].reshape(NL, D_IN).T)  # [4096, NL]
        Sl = S[:, rows]                                          # [M, NL]
        S_nm = np.ascontiguousarray(
            Sl.T.reshape(NT, 128, M).transpose(1, 0, 2)).astype(bf)
        m = dict(shared)
        m["xTb"] = np.ascontiguousarray(
            xT.reshape(KT, 128, NL)).astype(bf)
        m["S_nm"] = S_nm
        m["dinv1r"] = (Dinv[rows] / 4.0).reshape(1, NL).astype(bf)
        m["dinv2r"] = Dinv[rows].reshape(1, NL).astype(bf)
        in_maps.append(m)
    return in_maps


def _run(inputs, trace=False, tmpdir=None):
    global _PROGRAM
    _ensure_ntff_hook()
    from concourse.bass_utils import run_bass_kernel_spmd

    if _PROGRAM is None:
        _PROGRAM = build_program()
    in_maps = _prep_inputs(inputs)
    res = run_bass_kernel_spmd(_PROGRAM, in_maps, list(range(NCORES)),
                               trace=trace, tmpdir=tmpdir)
    out = np.empty((N, T, DM), np.float32)
    for c in range(NCORES):
        oT = np.asarray(res.results[c]["outT"], np.float32)  # [4096, NL]
        out[c * NL:(c + 1) * NL] = oT.T.reshape(NL, T, DM)
    return out, res


def kernel(**inputs) -> np.ndarray:
    out, _ = _run(inputs)
    return out


if __name__ == "__main__":
    d = np.load("/root/problem/inputs.npz")
    inp = {k: d[k] for k in d.files}
    got = kernel(**inp)
    exp = np.load("/root/problem/expected.npy")
    denom = np.abs(exp).max()
    print("rel err:", np.abs(got - exp).max() / denom)



# revision 23
# speedup vs baseline: 1.0588x; 1.0418x over previous
"""HGAT block on 8 Trainium2 NeuronCores (Bass/Tile, SPMD node-sharded).

Dense reformulation: the hypergraph incidence structure (he_nodes, he_edges)
is converted host-side into a dense per-core count matrix S[n_local, m]
(1024 x 1024), so every segment-sum becomes a dense matmul and the attention
softmax is computed on dense maps. Softmax denominators ride along as an
extra ones-column in the xt matmul. BatchNorm is shift-invariant per column,
so the conv biases bh1/bh2 drop out; BN affine corrections are applied
after the (pre-BN) he_attr matmul using he_attr(aff(h)) = aff(he_attr) with
the shift scaled by edge sizes Bm.

Sharding: nodes N=8192 split 1024/core. Per-edge partials (he_attr,
attention sums, eo aggregation) are all-reduced; BN stats ride as 2 extra
columns of the he_attr collectives for layers 1 and 2. x is loaded once in
bf16 (shared by the W1 matmul and the residual); output is stored bf16 and
upcast to f32 on host.

Master activations live in transposed layout hT[d=128 partitions, n=1024]
so BN affine/stats are per-partition ops; PE transposes flip orientation
where a matmul needs node-major operands.
"""

import sys
import types

import numpy as np

N, T, DM = 8192, 32, 128
M, NNZ = 1024, 131072
EPS = 1e-5
SLOPE = 0.2
NCORES = 8
NL = N // NCORES          # 1024 local nodes per core
NT = NL // 128            # 8 node tiles
MT = M // 128             # 8 edge tiles
KT = (T * DM) // 128      # 32 k-tiles for W1
D_IN = T * DM             # 4096

_PROGRAM = None


def _ensure_ntff_hook():
    try:
        import antenv.axon_hooks  # noqa: F401
        return
    except ImportError:
        pass
    try:
        import antenv
        from trn_agent_boot.trn_boot import _ntff_profile_via_ctypes
    except ImportError:
        return
    mod = types.ModuleType("antenv.axon_hooks")
    hook = _ntff_profile_via_ctypes("/opt/axon/libaxon_pjrt.so")
    mod.get_axon_ntff_profile_hook = lambda: hook
    mod.set_axon_ntff_profile_hook = lambda h: None
    sys.modules["antenv.axon_hooks"] = mod
    antenv.axon_hooks = mod


def build_program():
    from concourse import bacc, mybir, tile, masks

    f32 = mybir.dt.float32
    bf16 = mybir.dt.bfloat16
    AF = mybir.ActivationFunctionType
    ALU = mybir.AluOpType
    AX = mybir.AxisListType
    RG = [list(range(NCORES))]

    nc = bacc.Bacc("TRN2", target_bir_lowering=False, debug=False,
                   num_devices=NCORES)

    def din(name, shape, dt=f32):
        return nc.dram_tensor(name, list(shape), dt, kind="ExternalInput")

    xTb = din("xTb", [KT, 128, NL], bf16)
    S_nm = din("S_nm", [128, NT, M], bf16)
    W1b = din("W1b", [128, KT, DM], bf16)
    W3b = din("W3b", [DM, D_IN], bf16)
    Wh1b = din("Wh1b", [DM, 4 * DM], bf16)
    Wh2b = din("Wh2b", [DM, DM], bf16)
    wsx1 = din("wsx1", [DM, 4], bf16)
    wse1 = din("wse1", [DM, 4], bf16)
    wsx2 = din("wsx2", [DM, 1], bf16)
    wse2 = din("wse2", [DM, 1], bf16)
    b1T = din("b1T", [DM, 1])
    b3T = din("b3T", [DM, T])
    gbn = {k: din(k, [DM, 1]) for k in
           ("g1", "be1", "g2", "be2", "g3", "be3", "g4", "be4")}
    dinv1r = din("dinv1r", [1, NL], bf16)     # Dinv/heads as a row
    dinv2r = din("dinv2r", [1, NL], bf16)
    binvT = din("binvT", [128, MT])
    bmrow = din("bmrow", [1, M], bf16)
    outT = nc.dram_tensor("outT", [D_IN, NL], bf16, kind="ExternalOutput")

    GH = 2  # heads per attention group (bounds a0_em SBUF)

    ars1_in = nc.dram_tensor("ars1_in", [128, 2], f32)
    ars1_out = nc.dram_tensor("ars1_out", [128, 2], f32)
    ars2_in = nc.dram_tensor("ars2_in", [128, 2], f32)
    ars2_out = nc.dram_tensor("ars2_out", [128, 2], f32)
    he1_in = nc.dram_tensor("he1_in", [128, M], f32)
    he1_out = nc.dram_tensor("he1_out", [128, M], f32, addr_space="Shared")
    he2_in = nc.dram_tensor("he2_in", [128, M], f32)
    he2_out = nc.dram_tensor("he2_out", [128, M], f32, addr_space="Shared")
    eo1_in = nc.dram_tensor("eo1_in", [M, GH * 129], f32)
    eo1_out = nc.dram_tensor("eo1_out", [M, GH * 129], f32,
                             addr_space="Shared")
    eo2_in = nc.dram_tensor("eo2_in", [M, 129], f32)
    eo2_out = nc.dram_tensor("eo2_out", [M, 129], f32, addr_space="Shared")
    bn3_in = nc.dram_tensor("bn3_in", [128, 2], f32)
    bn3_out = nc.dram_tensor("bn3_out", [128, 2], f32)
    bn4_in = nc.dram_tensor("bn4_in", [128, 2], f32)
    bn4_out = nc.dram_tensor("bn4_out", [128, 2], f32)

    def allreduce(dst, src):
        nc.gpsimd.collective_compute(
            "AllReduce", ALU.add, replica_groups=RG,
            ins=[src[:].opt()], outs=[dst[:].opt()])

    with tile.TileContext(nc) as tc:
        with (
            tc.tile_pool(name="const", bufs=1) as constp,
            tc.tile_pool(name="state", bufs=1) as statep,
            tc.tile_pool(name="attn", bufs=1) as attnp,
            tc.tile_pool(name="big1", bufs=1) as big1p,
            tc.tile_pool(name="work", bufs=2) as workp,
            tc.tile_pool(name="xk", bufs=3) as xkp,
            tc.tile_pool(name="ps", bufs=4, space="PSUM") as psp,
            tc.tile_pool(name="ps_sm", bufs=4, space="PSUM") as pssm,
        ):
            ident = constp.tile([128, 128], bf16)
            masks.make_identity(nc, ident[:])

            def load_const(tag, shape, dt, src_ap, name=None):
                t = constp.tile(shape, dt, tag=tag, name=name or tag)
                nc.sync.dma_start(t[:], src_ap)
                return t

            # W1 and W3 share one slot (W3 loads after the W1 matmul);
            # the final-phase vB shares the S slot.
            w1_sb = load_const("wslot", [128, KT * DM], bf16,
                               W1b[:].rearrange("p k d -> p (k d)"),
                               name="w1sb")
            wh1_sb = load_const("wh1", [DM, 4 * DM], bf16, Wh1b[:])
            wh2_sb = load_const("wh2", [DM, DM], bf16, Wh2b[:])
            wsx1_sb = load_const("wsx1", [DM, 4], bf16, wsx1[:])
            wse1_sb = load_const("wse1", [DM, 4], bf16, wse1[:])
            wsx2_sb = load_const("wsx2", [DM, 1], bf16, wsx2[:])
            wse2_sb = load_const("wse2", [DM, 1], bf16, wse2[:])
            b1_sb = load_const("b1", [DM, 1], f32, b1T[:])
            b3_sb = load_const("b3", [DM, T], f32, b3T[:])
            gb_sb = {k: load_const(k, [DM, 1], f32, gbn[k][:]) for k in gbn}
            binv_sb = load_const("binv", [128, MT], f32, binvT[:])
            bm_row = load_const("bmr", [1, M], bf16, bmrow[:])
            d1_row = load_const("d1r", [1, NL], bf16, dinv1r[:])
            d2_row = load_const("d2r", [1, NL], bf16, dinv2r[:])
            eps_sb = constp.tile([128, 1], f32, tag="epsc")
            nc.gpsimd.memset(eps_sb[:], EPS)
            bm_bc = constp.tile([128, M], bf16, tag="bmbc")
            nc.gpsimd.partition_broadcast(bm_bc[:], bm_row[:1, :])
            dinv1_bc = constp.tile([128, NL], bf16, tag="d1bc")
            nc.gpsimd.partition_broadcast(dinv1_bc[:], d1_row[:1, :])
            dinv2_bc = constp.tile([128, NL], bf16, tag="d2bc")
            nc.gpsimd.partition_broadcast(dinv2_bc[:], d2_row[:1, :])
            s_sb = constp.tile([128, 2 * NT * M], bf16, tag="sslot",
                               name="ssb")
            nc.sync.dma_start(s_sb[:, 0:NT * M],
                              S_nm[:].rearrange("p n m -> p (n m)"))

            def s_tile(nt):
                return s_sb[:, nt * M:(nt + 1) * M]

            # transpose helper: quad-batched PE transposes, one DVE evac
            def transpose_cols(src_fn, dst, n128, dt=bf16):
                """dst[:, i*128:(i+1)*128] = src_fn(i).T for i in range(n128),
                batching 4 transposes per PSUM tile + single evac."""
                for q in range(0, n128, 4):
                    w = min(4, n128 - q)
                    trq = pssm.tile([128, 512], dt, tag="sm", name="trq")
                    for k in range(w):
                        nc.tensor.matmul(trq[:, k * 128:(k + 1) * 128],
                                         src_fn(q + k), ident[:],
                                         is_transpose=True)
                    nc.vector.tensor_copy(
                        dst[:, q * 128:(q + w) * 128], trq[:, 0:w * 128])

            # ======== h1 = lrelu(x @ W1 + b1), T-space ========
            hp = [psp.tile([128, 512], f32, tag="acc", name=f"w1p{i}")
                  for i in range(2)]
            for kt in range(KT):
                xk = xkp.tile([128, NL], bf16, tag="xk")
                nc.sync.dma_start(xk[:], xTb[kt, :, :])
                for i in range(2):
                    nc.tensor.matmul(
                        hp[i][:], w1_sb[:, kt * DM:(kt + 1) * DM],
                        xk[:, i * 512:(i + 1) * 512],
                        start=(kt == 0), stop=(kt == KT - 1))
            hT1 = statep.tile([128, NL], f32, tag="hT1")
            hT1_b = statep.tile([128, NL], bf16, tag="hT1b")
            for i in range(2):
                sl = slice(i * 512, (i + 1) * 512)
                nc.scalar.activation(hT1[:, sl], hp[i][:], AF.Prelu,
                                     bias=b1_sb[:, 0:1], alpha=SLOPE)
                nc.vector.tensor_copy(hT1_b[:, sl], hT1[:, sl])
            w3_sb = load_const("wslot", [DM, D_IN], bf16, W3b[:],
                               name="w3sb")

            ttr_dump = big1p.tile([128, NL], bf16, tag="ttrd")

            def stats_pair(st_ap, hT):
                """st cols 0/1 = sum(hT), sum(hT^2) along free axis."""
                nc.vector.reduce_sum(st_ap[:, 0:1], hT[:], axis=AX.X)
                nc.scalar.activation(ttr_dump[:], hT[:], AF.Square,
                                     accum_out=st_ap[:, 1:2])

            def bn_scales(sum_ap, sumsq_ap, g_sb, be_sb, count, tagp):
                sc = workp.tile([128, 1], f32, tag=f"sc{tagp}",
                                name=f"sc{tagp}")
                sh = workp.tile([128, 1], f32, tag=f"sh{tagp}",
                                name=f"sh{tagp}")
                tmp = workp.tile([128, 4], f32, tag="bnt", name=f"bnt{tagp}")
                mean, var, m2, rstd = (tmp[:, i:i + 1] for i in range(4))
                nc.scalar.mul(mean, sum_ap, 1.0 / count)
                nc.scalar.mul(var, sumsq_ap, 1.0 / count)
                nc.scalar.square(m2, mean)
                nc.vector.tensor_sub(var, var, m2)
                nc.scalar.activation(rstd, var, AF.Sqrt, bias=eps_sb[:, 0:1])
                nc.vector.reciprocal(rstd, rstd)
                nc.vector.tensor_mul(sc, g_sb[:], rstd)
                nc.vector.tensor_mul(sh, mean, sc)
                nc.vector.tensor_sub(sh, be_sb[:], sh)
                return sc, sh

            def bn_he_block(hT, hT_b, gk, bek, ars_in, ars_out, he_in,
                            he_out, heT_b, tagp):
                """Split collectives: stats allreduce (small, gates affine)
                runs first; the he_attr partial allreduce overlaps with the
                affine + downstream transposes."""
                st = workp.tile([128, 2], f32, tag="st", name=f"st{tagp}")
                stats_pair(st, hT)
                nc.sync.dma_start(ars_in[:], st[:])
                allreduce(ars_out, ars_in)
                # pre-BN node-major shadow for the he matmul
                hn_b = attnp.tile([128, NT * DM], bf16, tag="hnbn",
                                  name=f"hnbn{tagp}")
                transpose_cols(
                    lambda i: hT_b[:, i * 128:(i + 1) * 128], hn_b[:], NT)
                hep = [psp.tile([128, 512], f32, tag="acc", name=f"hep{i}")
                       for i in range(2)]
                for nt in range(NT):
                    for i in range(2):
                        nc.tensor.matmul(
                            hep[i][:], hn_b[:, nt * DM:(nt + 1) * DM],
                            s_tile(nt)[:, i * 512:(i + 1) * 512],
                            start=(nt == 0), stop=(nt == NT - 1))
                he_sb = big1p.tile([128, M], f32, tag="ar", name=f"he{tagp}")
                for i in range(2):
                    nc.vector.tensor_copy(he_sb[:, i * 512:(i + 1) * 512],
                                          hep[i][:])
                nc.sync.dma_start(he_in[:], he_sb[:])
                allreduce(he_out, he_in)
                str_ = workp.tile([128, 2], f32, tag="st", name=f"str{tagp}")
                nc.sync.dma_start(str_[:], ars_out[:])
                sc, sh = bn_scales(str_[:, 0:1], str_[:, 1:2],
                                   gb_sb[gk], gb_sb[bek], float(N), tagp)
                nc.vector.tensor_scalar(hT[:], hT[:], sc[:, 0:1], sh[:, 0:1],
                                        op0=ALU.mult, op1=ALU.add)
                nc.vector.tensor_copy(hT_b[:], hT[:])
                he_r = big1p.tile([128, M], f32, tag="ar", name=f"her{tagp}")
                nc.sync.dma_start(he_r[:], he_out[:])
                heTf = big1p.tile([128, M], f32, tag="heTf",
                                  name=f"heTf{tagp}")
                nc.vector.tensor_scalar(heTf[:], he_r[:], sc[:, 0:1],
                                        None, op0=ALU.mult)
                nc.vector.scalar_tensor_tensor(
                    heTf[:], bm_bc[:], sh[:, 0:1], heTf[:],
                    op0=ALU.mult, op1=ALU.add)
                nc.vector.tensor_copy(heT_b[:], heTf[:])

            heT1_b = statep.tile([128, M], bf16, tag="heT1b")
            bn_he_block(hT1, hT1_b, "g1", "be1", ars1_in, ars1_out,
                        he1_in, he1_out, heT1_b, "1")

            def hconv(heads, hT, hT_b, heT_b, wh_sb, wsx_sb, wse_sb,
                      dinv_bc, eo_in, eo_out, hT_new, hT_new_b, lname):
                C = 129
                hn_b = attnp.tile([128, NT * DM], bf16, tag="hnbn",
                                  name=f"hnc{lname}")
                transpose_cols(
                    lambda i: hT_b[:, i * 128:(i + 1) * 128], hn_b[:], NT)
                ax = attnp.tile([128, NT * 4], f32, tag="ax",
                                name=f"ax{lname}")
                for nt in range(NT):
                    aps = pssm.tile([128, 4], f32, tag="sm", name="axp")
                    nc.tensor.matmul(aps[:, 0:heads],
                                     hn_b[:, nt * DM:(nt + 1) * DM],
                                     wsx_sb[:], start=True, stop=True)
                    nc.vector.tensor_copy(ax[:, nt * 4:nt * 4 + heads],
                                          aps[:, 0:heads])
                ae_rows = []
                for h in range(heads):
                    ae_row = attnp.tile([1, M], bf16, tag="aerow", bufs=4,
                                        name=f"aerow{lname}{h}")
                    for i in range(2):
                        aep = pssm.tile([1, 512], f32, tag="sm", name="aep")
                        nc.tensor.matmul(aep[:], wse_sb[:, h:h + 1],
                                         heT_b[:, i * 512:(i + 1) * 512],
                                         start=True, stop=True)
                        nc.vector.tensor_copy(
                            ae_row[0:1, i * 512:(i + 1) * 512], aep[:])
                    ae_rows.append(ae_row)
                noTacc = attnp.tile([128, NL], f32, tag="noacc",
                                    name=f"noacc{lname}")
                ngroups = (heads + GH - 1) // GH
                for g in range(ngroups):
                    ghs = list(range(g * GH, min(heads, (g + 1) * GH)))
                    gh = len(ghs)
                    xta = attnp.tile([128, NT * GH * C], bf16, tag="xta",
                                     name=f"xta{lname}{g}")
                    nc.gpsimd.memset(xta[:], 1.0)
                    for nt in range(NT):
                        xps = psp.tile([128, 512], f32, tag="acc",
                                       name="xtp")
                        nc.tensor.matmul(
                            xps[:, 0:gh * DM],
                            hn_b[:, nt * DM:(nt + 1) * DM],
                            wh_sb[:, ghs[0] * DM:(ghs[0] + gh) * DM],
                            start=True, stop=True)
                        base = nt * GH * C
                        for j in range(gh):
                            nc.vector.tensor_copy(
                                xta[:, base + j * C:base + j * C + DM],
                                xps[:, j * DM:(j + 1) * DM])
                    a0_em = attnp.tile([128, GH * MT * NL], bf16, tag="a0em",
                                       name=f"a0em{lname}{g}")
                    for j, h in enumerate(ghs):
                        ae_bc = workp.tile([128, M], bf16, tag="aebc",
                                           name="aebc")
                        nc.gpsimd.partition_broadcast(ae_bc[:],
                                                      ae_rows[h][0:1, :])
                        a0_nm = workp.tile([128, NT * M], bf16, tag="a0nm",
                                           name="a0nm")
                        for nt in range(NT):
                            nc.scalar.activation(
                                a0_nm[:, nt * M:(nt + 1) * M], ae_bc[:],
                                AF.Prelu,
                                bias=ax[:, nt * 4 + h:nt * 4 + h + 1],
                                alpha=SLOPE)
                        for nt in range(NT):
                            zt = a0_nm[:, nt * M:(nt + 1) * M]
                            nc.scalar.activation(zt, zt, AF.Exp)
                        for nt in range(NT):
                            zt = a0_nm[:, nt * M:(nt + 1) * M]
                            nc.vector.tensor_mul(zt, zt, s_tile(nt))
                        for mt in range(MT):
                            eps_ = psp.tile([128, C], f32, tag="acc",
                                            name="eop")
                            for nt in range(NT):
                                nc.tensor.matmul(
                                    eps_[:],
                                    a0_nm[:, nt * M + mt * 128:
                                          nt * M + (mt + 1) * 128],
                                    xta[:, (nt * GH + j) * C:
                                        (nt * GH + j + 1) * C],
                                    start=(nt == 0), stop=(nt == NT - 1))
                            eo_st = workp.tile([128, C], f32, tag="eost",
                                               name="eost")
                            nc.vector.tensor_copy(eo_st[:], eps_[:])
                            nc.sync.dma_start(
                                eo_in[mt * 128:(mt + 1) * 128,
                                      j * C:(j + 1) * C], eo_st[:])
                        for mt in range(MT):
                            st_ = (j * MT + mt) * NL
                            transpose_cols(
                                lambda i, _mt=mt: a0_nm[
                                    :, i * M + _mt * 128:
                                    i * M + (_mt + 1) * 128],
                                a0_em[:, st_:st_ + NL], NT)
                    allreduce(eo_out, eo_in)
                    eoH = attnp.tile([128, MT * GH * DM], bf16, tag="eoH",
                                     name=f"eoH{lname}{g}")
                    for mt in range(MT):
                        eor = workp.tile([128, gh * C], f32, tag="eor",
                                         name="eor", bufs=2)
                        nc.sync.dma_start(
                            eor[:], eo_out[mt * 128:(mt + 1) * 128,
                                           0:gh * C])
                        for j in range(gh):
                            sm = workp.tile([128, 2], f32, tag="smt",
                                            name="smt")
                            nc.vector.tensor_scalar(
                                sm[:, 0:1], eor[:, j * C + DM:j * C + DM + 1],
                                1e-30, None, op0=ALU.max)
                            nc.vector.reciprocal(sm[:, 0:1], sm[:, 0:1])
                            nc.vector.tensor_mul(sm[:, 1:2], sm[:, 0:1],
                                                 binv_sb[:, mt:mt + 1])
                            nc.vector.tensor_mul(sm[:, 1:2], sm[:, 1:2],
                                                 sm[:, 0:1])
                            nc.vector.tensor_scalar(
                                eoH[:, (mt * GH + j) * DM:
                                    (mt * GH + j + 1) * DM],
                                eor[:, j * C:j * C + DM], sm[:, 1:2], None,
                                op0=ALU.mult)
                    noT = [psp.tile([128, 512], f32, tag="acc",
                                    name=f"noT{lname}{g}{i}")
                           for i in range(2)]
                    for half in range(2):
                        k = 0
                        for j in range(gh):
                            for mt in range(MT):
                                st_ = (j * MT + mt) * NL
                                nc.tensor.matmul(
                                    noT[half][:],
                                    eoH[:, (mt * GH + j) * DM:
                                        (mt * GH + j + 1) * DM],
                                    a0_em[:, st_ + half * 512:
                                          st_ + (half + 1) * 512],
                                    start=(k == 0),
                                    stop=(k == gh * MT - 1))
                                k += 1
                        sl = slice(half * 512, (half + 1) * 512)
                        if g == 0:
                            nc.vector.tensor_copy(noTacc[:, sl],
                                                  noT[half][:])
                        else:
                            nc.vector.tensor_add(noTacc[:, sl],
                                                 noTacc[:, sl], noT[half][:])
                # residual epilogue in T-space: hT_new = hT + noTacc * dinv
                nsc = workp.tile([128, NL], f32, tag="nsc", name="nsc")
                nc.vector.tensor_mul(nsc[:], noTacc[:], dinv_bc[:])
                nc.vector.tensor_add(hT_new[:], hT[:], nsc[:])
                nc.vector.tensor_copy(hT_new_b[:], hT_new[:])

            hT2 = statep.tile([128, NL], f32, tag="hT2")
            hT2_b = statep.tile([128, NL], bf16, tag="hT2b")
            hconv(4, hT1, hT1_b, heT1_b, wh1_sb, wsx1_sb, wse1_sb,
                  dinv1_bc, eo1_in, eo1_out, hT2, hT2_b, "A")

            heT2_b = statep.tile([128, M], bf16, tag="heT2b")
            bn_he_block(hT2, hT2_b, "g2", "be2", ars2_in, ars2_out,
                        he2_in, he2_out, heT2_b, "2")

            hT3 = statep.tile([128, NL], f32, tag="hT3")
            hT3_b = statep.tile([128, NL], bf16, tag="hT3b")
            hconv(1, hT2, hT2_b, heT2_b, wh2_sb, wsx2_sb, wse2_sb,
                  dinv2_bc, eo2_in, eo2_out, hT3, hT3_b, "B")

            # ---- BN3 ----
            st3 = workp.tile([128, 2], f32, tag="st", name="st3")
            stats_pair(st3, hT3)
            nc.sync.dma_start(bn3_in[:], st3[:])
            allreduce(bn3_out, bn3_in)
            st3r = workp.tile([128, 2], f32, tag="st", name="st3r")
            nc.sync.dma_start(st3r[:], bn3_out[:])
            sc3, sh3 = bn_scales(st3r[:, 0:1], st3r[:, 1:2],
                                 gb_sb["g3"], gb_sb["be3"], float(N), "3")
            nc.vector.tensor_scalar(hT3[:], hT3[:], sc3[:, 0:1], sh3[:, 0:1],
                                    op0=ALU.mult, op1=ALU.add)
            nc.vector.tensor_copy(hT3_b[:], hT3[:])

            # ---- y = lrelu(h3 @ W3 + b3); v = x + y (bf16); BN4 ----
            vA = attnp.tile([128, (KT // 2) * NL], bf16, tag="a0em",
                            name="vA")
            vB = constp.tile([128, (KT // 2) * NL], bf16, tag="sslot",
                             name="vB")

            def v_slice(jc):
                t = vA if jc < KT // 2 else vB
                j = jc if jc < KT // 2 else jc - KT // 2
                return t[:, j * NL:(j + 1) * NL]

            bst = workp.tile([128, KT * 2 * 6], f32, tag="vsum",
                             name="bst")
            for jc in range(KT):
                xres = xkp.tile([128, NL], bf16, tag="xk", name="xres")
                nc.sync.dma_start(xres[:], xTb[jc, :, :])
                ytmp = big1p.tile([128, NL], bf16, tag="vtmp", name="ytmp",
                                  bufs=2)
                for i in range(2):
                    sl = slice(i * 512, (i + 1) * 512)
                    yps = psp.tile([128, 512], f32, tag="acc", name="yp")
                    nc.tensor.matmul(yps[:], w3_sb[:, jc * 128:(jc + 1) * 128],
                                     hT3_b[:, sl], start=True, stop=True)
                    nc.scalar.activation(ytmp[:, sl], yps[:], AF.Prelu,
                                         bias=b3_sb[:, jc:jc + 1], alpha=SLOPE)
                nc.vector.tensor_add(v_slice(jc), ytmp[:], xres[:])
                for i in range(2):
                    nc.vector.bn_stats(
                        bst[:, (jc * 2 + i) * 6:(jc * 2 + i + 1) * 6],
                        v_slice(jc)[:, i * 512:(i + 1) * 512])
            # bn_aggr -> (mean, var); convert to (sum, sumsq) for the AR
            agg4 = workp.tile([128, 4], f32, tag="st", name="agg4")
            nc.vector.bn_aggr(agg4[:, 0:2], bst[:])
            st4s = workp.tile([128, 2], f32, tag="st", name="st4s")
            cnt = float(NL * KT)
            nc.scalar.mul(st4s[:, 0:1], agg4[:, 0:1], cnt)
            nc.scalar.square(agg4[:, 2:3], agg4[:, 0:1])
            nc.vector.tensor_add(agg4[:, 3:4], agg4[:, 1:2], agg4[:, 2:3])
            nc.scalar.mul(st4s[:, 1:2], agg4[:, 3:4], cnt)
            nc.sync.dma_start(bn4_in[:], st4s[:])
            allreduce(bn4_out, bn4_in)
            st4r = workp.tile([128, 2], f32, tag="st", name="st4r")
            nc.sync.dma_start(st4r[:], bn4_out[:])
            sc4, sh4 = bn_scales(st4r[:, 0:1], st4r[:, 1:2],
                                 gb_sb["g4"], gb_sb["be4"], float(N * T), "4")
            for jc in range(KT):
                ot = xkp.tile([128, NL], bf16, tag="xres", name="ot", bufs=3)
                nc.vector.tensor_scalar(ot[:], v_slice(jc),
                                        sc4[:, 0:1], sh4[:, 0:1],
                                        op0=ALU.mult, op1=ALU.add)
                nc.sync.dma_start(outT[jc * 128:(jc + 1) * 128, :], ot[:])

    nc.compile()
    return nc


def _prep_inputs(inputs):
    """Host-side preprocessing: shard, transpose, fold weights, build S."""
    x = np.ascontiguousarray(np.asarray(inputs["x"], np.float32))
    he_n = np.asarray(inputs["he_nodes"]).astype(np.int64)
    he_e = np.asarray(inputs["he_edges"]).astype(np.int64)
    W1 = np.asarray(inputs["W1"], np.float32)
    b1 = np.asarray(inputs["b1"], np.float32)
    Wh1 = np.asarray(inputs["Wh1"], np.float32)
    att1 = np.asarray(inputs["att1"], np.float32)
    Wh2 = np.asarray(inputs["Wh2"], np.float32)
    att2 = np.asarray(inputs["att2"], np.float32)
    W3 = np.asarray(inputs["W3"], np.float32)
    b3 = np.asarray(inputs["b3"], np.float32)

    try:
        import ml_dtypes
        bf = ml_dtypes.bfloat16
    except ImportError:  # pragma: no cover
        import jax.numpy as jnp
        bf = jnp.bfloat16

    S = np.zeros((M, N), np.float32)
    np.add.at(S, (he_e, he_n), 1.0)
    Dn = S.sum(axis=0)
    Bm = S.sum(axis=1)
    Dinv = np.where(Dn > 0, 1.0 / np.maximum(Dn, 1), 0.0).astype(np.float32)
    Binv = np.where(Bm > 0, 1.0 / np.maximum(Bm, 1), 0.0).astype(np.float32)

    def fold(Wh, att, heads):
        F = Wh.shape[1] // heads
        Whr = Wh.reshape(DM, heads, F)
        wx = np.einsum("dhf,hf->dh", Whr, att[0, :, :F]).astype(np.float32)
        we = np.einsum("dhf,hf->dh", Whr, att[0, :, F:]).astype(np.float32)
        return wx, we

    wx1, we1 = fold(Wh1, att1, 4)
    wx2, we2 = fold(Wh2, att2, 1)

    shared = {
        "W1b": np.ascontiguousarray(
            W1.reshape(KT, 128, DM).transpose(1, 0, 2)).astype(bf),
        "W3b": np.ascontiguousarray(W3).astype(bf),
        "Wh1b": np.ascontiguousarray(Wh1).astype(bf),
        "Wh2b": np.ascontiguousarray(Wh2).astype(bf),
        "wsx1": wx1.astype(bf), "wse1": we1.astype(bf),
        "wsx2": wx2.astype(bf), "wse2": we2.astype(bf),
        "b1T": b1.reshape(DM, 1),
        "b3T": np.ascontiguousarray(b3.reshape(T, DM).T),
        "binvT": np.ascontiguousarray(Binv.reshape(MT, 128).T),
        "bmrow": Bm.reshape(1, M).astype(bf),
    }
    for k in ("g1", "be1", "g2", "be2", "g3", "be3", "g4", "be4"):
        shared[k] = np.asarray(inputs[k], np.float32).reshape(DM, 1)

    in_maps = []
    for c in range(NCORES):
        rows = slice(c * NL, (c + 1) * NL)
        xT = np.ascontiguousarray(x[rows].reshape(NL, D_IN).T)  # [4096, NL]
        Sl = S[:, rows]                                          # [M, NL]
        S_nm = np.ascontiguousarray(
            Sl.T.reshape(NT, 128, M).transpose(1, 0, 2)).astype(bf)
        m = dict(shared)
        m["xTb"] = np.ascontiguousarray(
            xT.reshape(KT, 128, NL)).astype(bf)
        m["S_nm"] = S_nm
        m["dinv1r"] = (Dinv[rows] / 4.0).reshape(1, NL).astype(bf)
        m["dinv2r"] = Dinv[rows].reshape(1, NL).astype(bf)
        in_maps.append(m)
    return in_maps


def _run(inputs, trace=False, tmpdir=None):
    global _PROGRAM
    _ensure_ntff_hook()
    from concourse.bass_utils import run_bass_kernel_spmd

    if _PROGRAM is None:
        _PROGRAM = build_program()
    in_maps = _prep_inputs(inputs)
    res = run_bass_kernel_spmd(_PROGRAM, in_maps, list(range(NCORES)),
                               trace=trace, tmpdir=tmpdir)
    out = np.empty((N, T, DM), np.float32)
    for c in range(NCORES):
        oT = np.asarray(res.results[c]["outT"], np.float32)  # [4096, NL]
        out[c * NL:(c + 1) * NL] = oT.T.reshape(NL, T, DM)
    return out, res


def kernel(**inputs) -> np.ndarray:
    out, _ = _run(inputs)
    return out


if __name__ == "__main__":
    d = np.load("/root/problem/inputs.npz")
    inp = {k: d[k] for k in d.files}
    got = kernel(**inp)
    exp = np.load("/root/problem/expected.npy")
    denom = np.abs(exp).max()
    print("rel err:", np.abs(got - exp).max() / denom)



# revision 24
# speedup vs baseline: 1.0934x; 1.0326x over previous
"""HGAT block on 8 Trainium2 NeuronCores (Bass/Tile, SPMD node-sharded).

Dense reformulation: the hypergraph incidence structure (he_nodes, he_edges)
is converted host-side into a dense per-core count matrix S[n_local, m]
(1024 x 1024), so every segment-sum becomes a dense matmul and the attention
softmax is computed on dense maps. Softmax denominators ride along as an
extra ones-column in the xt matmul. BatchNorm is shift-invariant per column,
so the conv biases bh1/bh2 drop out; BN affine corrections are applied
after the (pre-BN) he_attr matmul using he_attr(aff(h)) = aff(he_attr) with
the shift scaled by edge sizes Bm.

Sharding: nodes N=8192 split 1024/core. Per-edge partials (he_attr,
attention sums, eo aggregation) are all-reduced; BN stats ride as 2 extra
columns of the he_attr collectives for layers 1 and 2. x is loaded once in
bf16 (shared by the W1 matmul and the residual); output is stored bf16 and
upcast to f32 on host.

Master activations live in transposed layout hT[d=128 partitions, n=1024]
so BN affine/stats are per-partition ops; PE transposes flip orientation
where a matmul needs node-major operands.
"""

import sys
import types

import numpy as np

N, T, DM = 8192, 32, 128
M, NNZ = 1024, 131072
EPS = 1e-5
SLOPE = 0.2
NCORES = 8
NL = N // NCORES          # 1024 local nodes per core
NT = NL // 128            # 8 node tiles
MT = M // 128             # 8 edge tiles
KT = (T * DM) // 128      # 32 k-tiles for W1
D_IN = T * DM             # 4096

_PROGRAM = None


def _ensure_ntff_hook():
    try:
        import antenv.axon_hooks  # noqa: F401
        return
    except ImportError:
        pass
    try:
        import antenv
        from trn_agent_boot.trn_boot import _ntff_profile_via_ctypes
    except ImportError:
        return
    mod = types.ModuleType("antenv.axon_hooks")
    hook = _ntff_profile_via_ctypes("/opt/axon/libaxon_pjrt.so")
    mod.get_axon_ntff_profile_hook = lambda: hook
    mod.set_axon_ntff_profile_hook = lambda h: None
    sys.modules["antenv.axon_hooks"] = mod
    antenv.axon_hooks = mod


def build_program():
    from concourse import bacc, mybir, tile, masks

    f32 = mybir.dt.float32
    bf16 = mybir.dt.bfloat16
    AF = mybir.ActivationFunctionType
    ALU = mybir.AluOpType
    AX = mybir.AxisListType
    RG = [list(range(NCORES))]

    nc = bacc.Bacc("TRN2", target_bir_lowering=False, debug=False,
                   num_devices=NCORES)

    def din(name, shape, dt=f32):
        return nc.dram_tensor(name, list(shape), dt, kind="ExternalInput")

    xTb = din("xTb", [KT, 128, NL], bf16)
    S_nm = din("S_nm", [128, NT, M], bf16)
    W1b = din("W1b", [128, KT, DM], bf16)
    W3b = din("W3b", [DM, D_IN], bf16)
    Wh1b = din("Wh1b", [DM, 4 * DM], bf16)
    Wh2b = din("Wh2b", [DM, DM], bf16)
    wsx1 = din("wsx1", [DM, 4], bf16)
    wse1 = din("wse1", [DM, 4], bf16)
    wsx2 = din("wsx2", [DM, 1], bf16)
    wse2 = din("wse2", [DM, 1], bf16)
    b1T = din("b1T", [DM, 1])
    b3T = din("b3T", [DM, T])
    gbn = {k: din(k, [DM, 1]) for k in
           ("g1", "be1", "g2", "be2", "g3", "be3", "g4", "be4")}
    dinv1r = din("dinv1r", [1, NL], bf16)     # Dinv/heads as a row
    dinv2r = din("dinv2r", [1, NL], bf16)
    binvT = din("binvT", [128, MT])
    bmrow = din("bmrow", [1, M], bf16)
    outT = nc.dram_tensor("outT", [D_IN, NL], bf16, kind="ExternalOutput")

    GH = 2  # heads per attention group (bounds a0_em SBUF)

    ars1_in = nc.dram_tensor("ars1_in", [128, 2], f32)
    ars1_out = nc.dram_tensor("ars1_out", [128, 2], f32)
    ars2_in = nc.dram_tensor("ars2_in", [128, 2], f32)
    ars2_out = nc.dram_tensor("ars2_out", [128, 2], f32)
    he1_in = nc.dram_tensor("he1_in", [128, M], f32)
    he1_out = nc.dram_tensor("he1_out", [128, M], f32, addr_space="Shared")
    he2_in = nc.dram_tensor("he2_in", [128, M], f32)
    he2_out = nc.dram_tensor("he2_out", [128, M], f32, addr_space="Shared")
    eo1_in = nc.dram_tensor("eo1_in", [M, GH * 129], f32)
    eo1_out = nc.dram_tensor("eo1_out", [M, GH * 129], f32,
                             addr_space="Shared")
    eo2_in = nc.dram_tensor("eo2_in", [M, 129], f32)
    eo2_out = nc.dram_tensor("eo2_out", [M, 129], f32, addr_space="Shared")
    bn3_in = nc.dram_tensor("bn3_in", [128, 2], f32)
    bn3_out = nc.dram_tensor("bn3_out", [128, 2], f32)
    bn4_in = nc.dram_tensor("bn4_in", [128, 2], f32)
    bn4_out = nc.dram_tensor("bn4_out", [128, 2], f32)

    def allreduce(dst, src):
        nc.gpsimd.collective_compute(
            "AllReduce", ALU.add, replica_groups=RG,
            ins=[src[:].opt()], outs=[dst[:].opt()])

    with tile.TileContext(nc) as tc:
        with (
            tc.tile_pool(name="const", bufs=1) as constp,
            tc.tile_pool(name="state", bufs=1) as statep,
            tc.tile_pool(name="attn", bufs=1) as attnp,
            tc.tile_pool(name="big1", bufs=1) as big1p,
            tc.tile_pool(name="work", bufs=2) as workp,
            tc.tile_pool(name="xk", bufs=3) as xkp,
            tc.tile_pool(name="ps", bufs=4, space="PSUM") as psp,
            tc.tile_pool(name="ps_sm", bufs=4, space="PSUM") as pssm,
        ):
            ident = constp.tile([128, 128], bf16)
            masks.make_identity(nc, ident[:])

            def load_const(tag, shape, dt, src_ap, name=None):
                t = constp.tile(shape, dt, tag=tag, name=name or tag)
                nc.sync.dma_start(t[:], src_ap)
                return t

            # W1 and W3 share one slot (W3 loads after the W1 matmul);
            # the final-phase vB shares the S slot.
            w1_sb = load_const("wslot", [128, KT * DM], bf16,
                               W1b[:].rearrange("p k d -> p (k d)"),
                               name="w1sb")
            wh1_sb = load_const("wh1", [DM, 4 * DM], bf16, Wh1b[:])
            wh2_sb = load_const("wh2", [DM, DM], bf16, Wh2b[:])
            wsx1_sb = load_const("wsx1", [DM, 4], bf16, wsx1[:])
            wse1_sb = load_const("wse1", [DM, 4], bf16, wse1[:])
            wsx2_sb = load_const("wsx2", [DM, 1], bf16, wsx2[:])
            wse2_sb = load_const("wse2", [DM, 1], bf16, wse2[:])
            b1_sb = load_const("b1", [DM, 1], f32, b1T[:])
            b3_sb = load_const("b3", [DM, T], f32, b3T[:])
            gb_sb = {k: load_const(k, [DM, 1], f32, gbn[k][:]) for k in gbn}
            binv_sb = load_const("binv", [128, MT], f32, binvT[:])
            bm_row = load_const("bmr", [1, M], bf16, bmrow[:])
            d1_row = load_const("d1r", [1, NL], bf16, dinv1r[:])
            d2_row = load_const("d2r", [1, NL], bf16, dinv2r[:])
            eps_sb = constp.tile([128, 1], f32, tag="epsc")
            nc.gpsimd.memset(eps_sb[:], EPS)
            bm_bc = constp.tile([128, M], bf16, tag="bmbc")
            nc.gpsimd.partition_broadcast(bm_bc[:], bm_row[:1, :])
            dinv1_bc = constp.tile([128, NL], bf16, tag="d1bc")
            nc.gpsimd.partition_broadcast(dinv1_bc[:], d1_row[:1, :])
            dinv2_bc = constp.tile([128, NL], bf16, tag="d2bc")
            nc.gpsimd.partition_broadcast(dinv2_bc[:], d2_row[:1, :])
            s_sb = constp.tile([128, 2 * NT * M], bf16, tag="sslot",
                               name="ssb")
            nc.sync.dma_start(s_sb[:, 0:NT * M],
                              S_nm[:].rearrange("p n m -> p (n m)"))

            def s_tile(nt):
                return s_sb[:, nt * M:(nt + 1) * M]

            # transpose helper: quad-batched PE transposes, one DVE evac
            def transpose_cols(src_fn, dst, n128, dt=bf16):
                """dst[:, i*128:(i+1)*128] = src_fn(i).T for i in range(n128),
                batching 4 transposes per PSUM tile + single evac."""
                for q in range(0, n128, 4):
                    w = min(4, n128 - q)
                    trq = pssm.tile([128, 512], dt, tag="sm", name="trq")
                    for k in range(w):
                        nc.tensor.matmul(trq[:, k * 128:(k + 1) * 128],
                                         src_fn(q + k), ident[:],
                                         is_transpose=True)
                    nc.vector.tensor_copy(
                        dst[:, q * 128:(q + w) * 128], trq[:, 0:w * 128])

            # ======== h1 = lrelu(x @ W1 + b1), T-space ========
            hp = [psp.tile([128, 512], f32, tag="acc", name=f"w1p{i}")
                  for i in range(2)]
            for kt in range(KT):
                xk = xkp.tile([128, NL], bf16, tag="xk")
                nc.sync.dma_start(xk[:], xTb[kt, :, :])
                for i in range(2):
                    nc.tensor.matmul(
                        hp[i][:], w1_sb[:, kt * DM:(kt + 1) * DM],
                        xk[:, i * 512:(i + 1) * 512],
                        start=(kt == 0), stop=(kt == KT - 1))
            hT1 = statep.tile([128, NL], f32, tag="hT1")
            hT1_b = statep.tile([128, NL], bf16, tag="hT1b")
            for i in range(2):
                sl = slice(i * 512, (i + 1) * 512)
                nc.scalar.activation(hT1[:, sl], hp[i][:], AF.Prelu,
                                     bias=b1_sb[:, 0:1], alpha=SLOPE)
                nc.vector.tensor_copy(hT1_b[:, sl], hT1[:, sl])
            w3_sb = load_const("wslot", [DM, D_IN], bf16, W3b[:],
                               name="w3sb")

            ttr_dump = big1p.tile([128, NL], bf16, tag="ttrd")

            def stats_pair(st_ap, hT):
                """st cols 0/1 = sum(hT), sum(hT^2) along free axis."""
                nc.vector.reduce_sum(st_ap[:, 0:1], hT[:], axis=AX.X)
                nc.scalar.activation(ttr_dump[:], hT[:], AF.Square,
                                     accum_out=st_ap[:, 1:2])

            def bn_scales(sum_ap, sumsq_ap, g_sb, be_sb, count, tagp):
                sc = workp.tile([128, 1], f32, tag=f"sc{tagp}",
                                name=f"sc{tagp}")
                sh = workp.tile([128, 1], f32, tag=f"sh{tagp}",
                                name=f"sh{tagp}")
                tmp = workp.tile([128, 4], f32, tag="bnt", name=f"bnt{tagp}")
                mean, var, m2, rstd = (tmp[:, i:i + 1] for i in range(4))
                nc.scalar.mul(mean, sum_ap, 1.0 / count)
                nc.scalar.mul(var, sumsq_ap, 1.0 / count)
                nc.scalar.square(m2, mean)
                nc.vector.tensor_sub(var, var, m2)
                nc.scalar.activation(rstd, var, AF.Sqrt, bias=eps_sb[:, 0:1])
                nc.vector.reciprocal(rstd, rstd)
                nc.vector.tensor_mul(sc, g_sb[:], rstd)
                nc.vector.tensor_mul(sh, mean, sc)
                nc.vector.tensor_sub(sh, be_sb[:], sh)
                return sc, sh

            def bn_he_block(hT, hT_b, gk, bek, ars_in, ars_out, he_in,
                            he_out, heT_b, tagp):
                """Split collectives: stats allreduce (small, gates affine)
                runs first; the he_attr partial allreduce overlaps with the
                affine + downstream transposes."""
                st = workp.tile([128, 2], f32, tag="st", name=f"st{tagp}")
                stats_pair(st, hT)
                nc.sync.dma_start(ars_in[:], st[:])
                allreduce(ars_out, ars_in)
                # pre-BN node-major shadow for the he matmul
                hn_b = attnp.tile([128, NT * DM], bf16, tag="hnbn",
                                  name=f"hnbn{tagp}")
                transpose_cols(
                    lambda i: hT_b[:, i * 128:(i + 1) * 128], hn_b[:], NT)
                hep = [psp.tile([128, 512], f32, tag="acc", name=f"hep{i}")
                       for i in range(2)]
                for nt in range(NT):
                    for i in range(2):
                        nc.tensor.matmul(
                            hep[i][:], hn_b[:, nt * DM:(nt + 1) * DM],
                            s_tile(nt)[:, i * 512:(i + 1) * 512],
                            start=(nt == 0), stop=(nt == NT - 1))
                he_sb = big1p.tile([128, M], f32, tag="ar", name=f"he{tagp}")
                for i in range(2):
                    nc.vector.tensor_copy(he_sb[:, i * 512:(i + 1) * 512],
                                          hep[i][:])
                nc.sync.dma_start(he_in[:], he_sb[:])
                allreduce(he_out, he_in)
                str_ = workp.tile([128, 2], f32, tag="st", name=f"str{tagp}")
                nc.sync.dma_start(str_[:], ars_out[:])
                sc, sh = bn_scales(str_[:, 0:1], str_[:, 1:2],
                                   gb_sb[gk], gb_sb[bek], float(N), tagp)
                nc.vector.tensor_scalar(hT[:], hT[:], sc[:, 0:1], sh[:, 0:1],
                                        op0=ALU.mult, op1=ALU.add)
                nc.vector.tensor_copy(hT_b[:], hT[:])
                he_r = big1p.tile([128, M], f32, tag="ar", name=f"her{tagp}")
                nc.sync.dma_start(he_r[:], he_out[:])
                heTf = big1p.tile([128, M], f32, tag="heTf",
                                  name=f"heTf{tagp}")
                nc.vector.tensor_scalar(heTf[:], he_r[:], sc[:, 0:1],
                                        None, op0=ALU.mult)
                nc.vector.scalar_tensor_tensor(
                    heTf[:], bm_bc[:], sh[:, 0:1], heTf[:],
                    op0=ALU.mult, op1=ALU.add)
                nc.vector.tensor_copy(heT_b[:], heTf[:])

            heT1_b = statep.tile([128, M], bf16, tag="heT1b")
            bn_he_block(hT1, hT1_b, "g1", "be1", ars1_in, ars1_out,
                        he1_in, he1_out, heT1_b, "1")

            def hconv(heads, hT, hT_b, heT_b, wh_sb, wsx_sb, wse_sb,
                      dinv_bc, eo_in, eo_out, hT_new, hT_new_b, lname):
                C = 129
                hn_b = attnp.tile([128, NT * DM], bf16, tag="hnbn",
                                  name=f"hnc{lname}")
                transpose_cols(
                    lambda i: hT_b[:, i * 128:(i + 1) * 128], hn_b[:], NT)
                ax = attnp.tile([128, NT * 4], f32, tag="ax",
                                name=f"ax{lname}")
                for nt in range(NT):
                    aps = pssm.tile([128, 4], f32, tag="sm", name="axp")
                    nc.tensor.matmul(aps[:, 0:heads],
                                     hn_b[:, nt * DM:(nt + 1) * DM],
                                     wsx_sb[:], start=True, stop=True)
                    nc.vector.tensor_copy(ax[:, nt * 4:nt * 4 + heads],
                                          aps[:, 0:heads])
                ae_rows = []
                for h in range(heads):
                    ae_row = attnp.tile([1, M], bf16, tag="aerow", bufs=4,
                                        name=f"aerow{lname}{h}")
                    for i in range(2):
                        aep = pssm.tile([1, 512], f32, tag="sm", name="aep")
                        nc.tensor.matmul(aep[:], wse_sb[:, h:h + 1],
                                         heT_b[:, i * 512:(i + 1) * 512],
                                         start=True, stop=True)
                        nc.vector.tensor_copy(
                            ae_row[0:1, i * 512:(i + 1) * 512], aep[:])
                    ae_rows.append(ae_row)
                noTacc = attnp.tile([128, NL], f32, tag="noacc",
                                    name=f"noacc{lname}")
                ngroups = (heads + GH - 1) // GH
                for g in range(ngroups):
                    ghs = list(range(g * GH, min(heads, (g + 1) * GH)))
                    gh = len(ghs)
                    xta = attnp.tile([128, NT * GH * C], bf16, tag="xta",
                                     name=f"xta{lname}{g}")
                    nc.gpsimd.memset(xta[:], 1.0)
                    for nt in range(NT):
                        xps = psp.tile([128, 512], f32, tag="acc",
                                       name="xtp")
                        nc.tensor.matmul(
                            xps[:, 0:gh * DM],
                            hn_b[:, nt * DM:(nt + 1) * DM],
                            wh_sb[:, ghs[0] * DM:(ghs[0] + gh) * DM],
                            start=True, stop=True)
                        base = nt * GH * C
                        for j in range(gh):
                            nc.vector.tensor_copy(
                                xta[:, base + j * C:base + j * C + DM],
                                xps[:, j * DM:(j + 1) * DM])
                    a0_em = attnp.tile([128, GH * MT * NL], bf16, tag="a0em",
                                       name=f"a0em{lname}{g}")
                    for j, h in enumerate(ghs):
                        ae_bc = workp.tile([128, M], bf16, tag="aebc",
                                           name="aebc")
                        nc.gpsimd.partition_broadcast(ae_bc[:],
                                                      ae_rows[h][0:1, :])
                        a0_nm = workp.tile([128, NT * M], bf16, tag="a0nm",
                                           name="a0nm")
                        for nt in range(NT):
                            nc.scalar.activation(
                                a0_nm[:, nt * M:(nt + 1) * M], ae_bc[:],
                                AF.Prelu,
                                bias=ax[:, nt * 4 + h:nt * 4 + h + 1],
                                alpha=SLOPE)
                        for nt in range(NT):
                            zt = a0_nm[:, nt * M:(nt + 1) * M]
                            nc.scalar.activation(zt, zt, AF.Exp)
                        for nt in range(NT):
                            zt = a0_nm[:, nt * M:(nt + 1) * M]
                            nc.vector.tensor_mul(zt, zt, s_tile(nt))
                        for mt in range(MT):
                            eps_ = psp.tile([128, C], f32, tag="acc",
                                            name="eop")
                            for nt in range(NT):
                                nc.tensor.matmul(
                                    eps_[:],
                                    a0_nm[:, nt * M + mt * 128:
                                          nt * M + (mt + 1) * 128],
                                    xta[:, (nt * GH + j) * C:
                                        (nt * GH + j + 1) * C],
                                    start=(nt == 0), stop=(nt == NT - 1))
                            eo_st = workp.tile([128, C], f32, tag="eost",
                                               name="eost")
                            nc.vector.tensor_copy(eo_st[:], eps_[:])
                            nc.sync.dma_start(
                                eo_in[mt * 128:(mt + 1) * 128,
                                      j * C:(j + 1) * C], eo_st[:])
                        for mt in range(MT):
                            st_ = (j * MT + mt) * NL
                            transpose_cols(
                                lambda i, _mt=mt: a0_nm[
                                    :, i * M + _mt * 128:
                                    i * M + (_mt + 1) * 128],
                                a0_em[:, st_:st_ + NL], NT)
                    allreduce(eo_out, eo_in)
                    eoH = attnp.tile([128, MT * GH * DM], bf16, tag="eoH",
                                     name=f"eoH{lname}{g}")
                    for mt in range(MT):
                        eor = workp.tile([128, gh * C], f32, tag="eor",
                                         name="eor", bufs=2)
                        nc.sync.dma_start(
                            eor[:], eo_out[mt * 128:(mt + 1) * 128,
                                           0:gh * C])
                        for j in range(gh):
                            sm = workp.tile([128, 2], f32, tag="smt",
                                            name="smt")
                            nc.vector.tensor_scalar(
                                sm[:, 0:1], eor[:, j * C + DM:j * C + DM + 1],
                                1e-30, None, op0=ALU.max)
                            nc.vector.reciprocal(sm[:, 0:1], sm[:, 0:1])
                            nc.vector.tensor_mul(sm[:, 1:2], sm[:, 0:1],
                                                 binv_sb[:, mt:mt + 1])
                            nc.vector.tensor_mul(sm[:, 1:2], sm[:, 1:2],
                                                 sm[:, 0:1])
                            nc.vector.tensor_scalar(
                                eoH[:, (mt * GH + j) * DM:
                                    (mt * GH + j + 1) * DM],
                                eor[:, j * C:j * C + DM], sm[:, 1:2], None,
                                op0=ALU.mult)
                    noT = [psp.tile([128, 512], f32, tag="acc",
                                    name=f"noT{lname}{g}{i}")
                           for i in range(2)]
                    for half in range(2):
                        k = 0
                        for j in range(gh):
                            for mt in range(MT):
                                st_ = (j * MT + mt) * NL
                                nc.tensor.matmul(
                                    noT[half][:],
                                    eoH[:, (mt * GH + j) * DM:
                                        (mt * GH + j + 1) * DM],
                                    a0_em[:, st_ + half * 512:
                                          st_ + (half + 1) * 512],
                                    start=(k == 0),
                                    stop=(k == gh * MT - 1))
                                k += 1
                        sl = slice(half * 512, (half + 1) * 512)
                        if g == 0:
                            nc.vector.tensor_copy(noTacc[:, sl],
                                                  noT[half][:])
                        else:
                            nc.vector.tensor_add(noTacc[:, sl],
                                                 noTacc[:, sl], noT[half][:])
                # residual epilogue in T-space: hT_new = hT + noTacc * dinv
                nsc = workp.tile([128, NL], f32, tag="nsc", name="nsc")
                nc.vector.tensor_mul(nsc[:], noTacc[:], dinv_bc[:])
                nc.vector.tensor_add(hT_new[:], hT[:], nsc[:])
                nc.vector.tensor_copy(hT_new_b[:], hT_new[:])

            hT2 = statep.tile([128, NL], f32, tag="hT2")
            hT2_b = statep.tile([128, NL], bf16, tag="hT2b")
            hconv(4, hT1, hT1_b, heT1_b, wh1_sb, wsx1_sb, wse1_sb,
                  dinv1_bc, eo1_in, eo1_out, hT2, hT2_b, "A")

            heT2_b = statep.tile([128, M], bf16, tag="heT2b")
            bn_he_block(hT2, hT2_b, "g2", "be2", ars2_in, ars2_out,
                        he2_in, he2_out, heT2_b, "2")

            hT3 = statep.tile([128, NL], f32, tag="hT3")
            hT3_b = statep.tile([128, NL], bf16, tag="hT3b")
            hconv(1, hT2, hT2_b, heT2_b, wh2_sb, wsx2_sb, wse2_sb,
                  dinv2_bc, eo2_in, eo2_out, hT3, hT3_b, "B")

            # ---- BN3 ----
            st3 = workp.tile([128, 2], f32, tag="st", name="st3")
            stats_pair(st3, hT3)
            nc.sync.dma_start(bn3_in[:], st3[:])
            allreduce(bn3_out, bn3_in)
            st3r = workp.tile([128, 2], f32, tag="st", name="st3r")
            nc.sync.dma_start(st3r[:], bn3_out[:])
            sc3, sh3 = bn_scales(st3r[:, 0:1], st3r[:, 1:2],
                                 gb_sb["g3"], gb_sb["be3"], float(N), "3")
            nc.vector.tensor_scalar(hT3[:], hT3[:], sc3[:, 0:1], sh3[:, 0:1],
                                    op0=ALU.mult, op1=ALU.add)
            nc.vector.tensor_copy(hT3_b[:], hT3[:])

            # ---- y = lrelu(h3 @ W3 + b3); v = x + y (bf16); BN4 ----
            vA = attnp.tile([128, (KT // 2) * NL], bf16, tag="a0em",
                            name="vA")
            vB = constp.tile([128, (KT // 2) * NL], bf16, tag="sslot",
                             name="vB")

            def v_slice(jc):
                t = vA if jc < KT // 2 else vB
                j = jc if jc < KT // 2 else jc - KT // 2
                return t[:, j * NL:(j + 1) * NL]

            vsum = workp.tile([128, KT], f32, tag="vsum", name="vsum")
            vsq = workp.tile([128, KT], f32, tag="vsq", name="vsq")
            for jc in range(KT):
                xres = xkp.tile([128, NL], bf16, tag="xk", name="xres")
                nc.sync.dma_start(xres[:], xTb[jc, :, :])
                ytmp = big1p.tile([128, NL], bf16, tag="vtmp", name="ytmp",
                                  bufs=2)
                for i in range(2):
                    sl = slice(i * 512, (i + 1) * 512)
                    yps = psp.tile([128, 512], f32, tag="acc", name="yp")
                    nc.tensor.matmul(yps[:], w3_sb[:, jc * 128:(jc + 1) * 128],
                                     hT3_b[:, sl], start=True, stop=True)
                    nc.scalar.activation(ytmp[:, sl], yps[:], AF.Prelu,
                                         bias=b3_sb[:, jc:jc + 1], alpha=SLOPE)
                nc.vector.tensor_add(v_slice(jc), ytmp[:], xres[:])
                nc.vector.reduce_sum(vsum[:, jc:jc + 1], v_slice(jc),
                                     axis=AX.X)
                nc.scalar.activation(ttr_dump[:], v_slice(jc), AF.Square,
                                     accum_out=vsq[:, jc:jc + 1])
            st4s = workp.tile([128, 2], f32, tag="st", name="st4s")
            nc.vector.reduce_sum(st4s[:, 0:1], vsum[:], axis=AX.X)
            nc.vector.reduce_sum(st4s[:, 1:2], vsq[:], axis=AX.X)
            nc.sync.dma_start(bn4_in[:], st4s[:])
            allreduce(bn4_out, bn4_in)
            st4r = workp.tile([128, 2], f32, tag="st", name="st4r")
            nc.sync.dma_start(st4r[:], bn4_out[:])
            sc4, sh4 = bn_scales(st4r[:, 0:1], st4r[:, 1:2],
                                 gb_sb["g4"], gb_sb["be4"], float(N * T), "4")
            for jc in range(KT):
                ot = xkp.tile([128, NL], bf16, tag="xres", name="ot", bufs=3)
                nc.vector.tensor_scalar(ot[:], v_slice(jc),
                                        sc4[:, 0:1], sh4[:, 0:1],
                                        op0=ALU.mult, op1=ALU.add)
                nc.sync.dma_start(outT[jc * 128:(jc + 1) * 128, :], ot[:])

    nc.compile()
    return nc


def _prep_inputs(inputs):
    """Host-side preprocessing: shard, transpose, fold weights, build S."""
    x = np.ascontiguousarray(np.asarray(inputs["x"], np.float32))
    he_n = np.asarray(inputs["he_nodes"]).astype(np.int64)
    he_e = np.asarray(inputs["he_edges"]).astype(np.int64)
    W1 = np.asarray(inputs["W1"], np.float32)
    b1 = np.asarray(inputs["b1"], np.float32)
    Wh1 = np.asarray(inputs["Wh1"], np.float32)
    att1 = np.asarray(inputs["att1"], np.float32)
    Wh2 = np.asarray(inputs["Wh2"], np.float32)
    att2 = np.asarray(inputs["att2"], np.float32)
    W3 = np.asarray(inputs["W3"], np.float32)
    b3 = np.asarray(inputs["b3"], np.float32)

    try:
        import ml_dtypes
        bf = ml_dtypes.bfloat16
    except ImportError:  # pragma: no cover
        import jax.numpy as jnp
        bf = jnp.bfloat16

    S = np.zeros((M, N), np.float32)
    np.add.at(S, (he_e, he_n), 1.0)
    Dn = S.sum(axis=0)
    Bm = S.sum(axis=1)
    Dinv = np.where(Dn > 0, 1.0 / np.maximum(Dn, 1), 0.0).astype(np.float32)
    Binv = np.where(Bm > 0, 1.0 / np.maximum(Bm, 1), 0.0).astype(np.float32)

    def fold(Wh, att, heads):
        F = Wh.shape[1] // heads
        Whr = Wh.reshape(DM, heads, F)
        wx = np.einsum("dhf,hf->dh", Whr, att[0, :, :F]).astype(np.float32)
        we = np.einsum("dhf,hf->dh", Whr, att[0, :, F:]).astype(np.float32)
        return wx, we

    wx1, we1 = fold(Wh1, att1, 4)
    wx2, we2 = fold(Wh2, att2, 1)

    shared = {
        "W1b": np.ascontiguousarray(
            W1.reshape(KT, 128, DM).transpose(1, 0, 2)).astype(bf),
        "W3b": np.ascontiguousarray(W3).astype(bf),
        "Wh1b": np.ascontiguousarray(Wh1).astype(bf),
        "Wh2b": np.ascontiguousarray(Wh2).astype(bf),
        "wsx1": wx1.astype(bf), "wse1": we1.astype(bf),
        "wsx2": wx2.astype(bf), "wse2": we2.astype(bf),
        "b1T": b1.reshape(DM, 1),
        "b3T": np.ascontiguousarray(b3.reshape(T, DM).T),
        "binvT": np.ascontiguousarray(Binv.reshape(MT, 128).T),
        "bmrow": Bm.reshape(1, M).astype(bf),
    }
    for k in ("g1", "be1", "g2", "be2", "g3", "be3", "g4", "be4"):
        shared[k] = np.asarray(inputs[k], np.float32).reshape(DM, 1)

    in_maps = []
    for c in range(NCORES):
        rows = slice(c * NL, (c + 1) * NL)
        xT = np.ascontiguousarray(x[rows].reshape(NL, D_IN).T)  # [4096, NL]
        Sl = S[:, rows]                                          # [M, NL]
        S_nm = np.ascontiguousarray(
            Sl.T.reshape(NT, 128, M).transpose(1, 0, 2)).astype(bf)
        m = dict(shared)
        m["xTb"] = np.ascontiguousarray(
            xT.reshape(KT, 128, NL)).astype(bf)
        m["S_nm"] = S_nm
        m["dinv1r"] = (Dinv[rows] / 4.0).reshape(1, NL).astype(bf)
        m["dinv2r"] = Dinv[rows].reshape(1, NL).astype(bf)
        in_maps.append(m)
    return in_maps


def _run(inputs, trace=False, tmpdir=None):
    global _PROGRAM
    _ensure_ntff_hook()
    from concourse.bass_utils import run_bass_kernel_spmd

    if _PROGRAM is None:
        _PROGRAM = build_program()
    in_maps = _prep_inputs(inputs)
    res = run_bass_kernel_spmd(_PROGRAM, in_maps, list(range(NCORES)),
                               trace=trace, tmpdir=tmpdir)
    out = np.empty((N, T, DM), np.float32)
    for c in range(NCORES):
        oT = np.asarray(res.results[c]["outT"], np.float32)  # [4096, NL]
        out[c * NL:(c + 1) * NL] = oT.T.reshape(NL, T, DM)
    return out, res


def kernel(**inputs) -> np.ndarray:
    out, _ = _run(inputs)
    return out


if __name__ == "__main__":
    d = np.load("/root/problem/inputs.npz")
    inp = {k: d[k] for k in d.files}
    got = kernel(**inp)
    exp = np.load("/root/problem/expected.npy")
    denom = np.abs(exp).max()
    print("rel err:", np.abs(got - exp).max() / denom)

